# revision 58
# baseline (speedup 1.0000x reference)
"""Trainium2 Bass kernel for nn_CrossfusionBidirectional.

Sharding: 8 cores = (batch b in {0,1}) x (query-row quarter qi in {0..3}).
Each core computes output rows [qi*784, (qi+1)*784) of batch b with zero
cross-core communication; the host concatenates the 8 slices.

v2: bf16 dataflow. All weights and SBUF activations are bf16 (PSUM stays
f32, LayerNorm statistics in f32); the pre-attention stages run as one
merged per-chunk pipeline with p2up/pp kept in SBUF (no DRAM roundtrip).
Attention is computed transposed (S^T[j, q]) with multiplicative rel-pos
bias exp(s + kb) * exp(B); softmax denominators come from an all-ones
matmul whose output is already broadcast across partitions. LayerNorm
affine params and gammas are folded into downstream weights on the host;
K-projection biases drop out via softmax shift invariance; Q-projection
biases enter through the per-key exp bias column; V-projection biases fold
into the output-projection bias because softmax rows sum to one.
"""

import numpy as np

B, L, C, HEADS = 2, 3136, 384, 3
H, H2 = 56, 28
L2 = L // 4
HD = C // HEADS
EPS = 1e-5
NCORES = 8
QPC = L // 4          # 784 query rows per core
CT = C // 128         # 3 feature tiles
NMC, MC = 8, 392      # merged-loop chunking of full L
NQC, QC = 2, 392      # per-core query chunking
TOK2, TT2 = 7, 112    # low-res token tiling (784 = 7*112)
JTS = [(i * 128, 128) for i in range(24)] + [(3072, 64)]   # key tiles
EBG = [(0, 5), (5, 10), (10, 15), (15, 20), (20, 24), (24, 25)]  # eb DMA groups

_COMPILED = None


def _resize_weight_mat(n_in, n_out):
    # jax.image.resize 'linear' half-pixel: triangle kernel, normalized
    scale = n_out / n_in
    sample_f = (np.arange(n_out) + 0.5) / scale - 0.5
    w = 1.0 - np.abs(sample_f[:, None] - np.arange(n_in)[None, :])
    w = np.clip(w, 0.0, 1.0)
    w = w / w.sum(axis=1, keepdims=True)
    return w.astype(np.float32)


def _host_prep(inp):
    f32 = np.float32
    g = {}
    scale = f32(HD ** -0.5)
    n1w, n1b = inp["n1_w"].astype(f32), inp["n1_b"].astype(f32)
    n2w, n2b = inp["n2_w"].astype(f32), inp["n2_b"].astype(f32)

    def fold_in(w, b, lnw, lnb):
        return (w * lnw[None, :]).astype(f32), (b + w @ lnb).astype(f32)

    wqh, bqh = fold_in(inp["wqh_w"], inp["wqh_b"], n2w, n2b)
    wkh, _ = fold_in(inp["wkh_w"], inp["wkh_b"], n1w, n1b)
    wvh, bvh = fold_in(inp["wvh_w"], inp["wvh_b"], n1w, n1b)
    wql, bql = fold_in(inp["wql_w"], inp["wql_b"], n1w, n1b)
    wkl = inp["wkl_w"].astype(f32)
    wvl, bvl = inp["wvl_w"].astype(f32), inp["wvl_b"].astype(f32)

    g["wqhT"], g["bqh"] = (wqh.T * scale).copy(), bqh * scale
    g["wqlT"], g["bql"] = (wql.T * scale).copy(), bql * scale
    g["wkhT"], g["wklT"] = wkh.T.copy(), wkl.T.copy()
    g["wvhT"], g["wvlT"] = wvh.T.copy(), wvl.T.copy()

    pl1L, pl1R = inp["pl1_w"][:, :C], inp["pl1_w"][:, C:]
    pl1Lw, _ = fold_in(pl1L, np.zeros(C, f32), n2w, n2b)
    pl1Rw, _ = fold_in(pl1R, np.zeros(C, f32), n1w, n1b)
    g["pl1LT"], g["pl1RT"] = pl1Lw.T.copy(), pl1Rw.T.copy()
    g["pl1b"] = (inp["pl1_b"] + pl1L @ n2b + pl1R @ n1b).astype(f32)
    g["pl2T"], g["pl2b"] = inp["pl2_w"].T.copy(), inp["pl2_b"].astype(f32)

    gh, gl = f32(inp["gamma_h"][0]), f32(inp["gamma_l"][0])
    g["fohT"] = (inp["foh_w"].T * gh).astype(f32)
    g["fohb"] = ((inp["foh_b"] + inp["foh_w"] @ bvh) * gh).astype(f32)
    g["folT"] = (inp["fol_w"].T * gl).astype(f32)
    g["folb"] = ((inp["fol_b"] + inp["fol_w"] @ bvl) * gl).astype(f32)

    g["g1LT"] = inp["g1_w"][:, :C].T.copy().astype(f32)
    g["g1RT"] = inp["g1_w"][:, C:].T.copy().astype(f32)
    g["g1b"] = inp["g1_b"].astype(f32)
    g["g2T"] = inp["g2_w"].T.copy().astype(f32)   # [384, 1]
    g["g2b"] = inp["g2_b"].astype(f32)            # [1]

    ffL, ffR = inp["ff_w"][:, :C], inp["ff_w"][:, C:]
    g["ffLT"] = ffL.T.copy().astype(f32)
    g["ffPT"] = (ffL + ffR).T.copy().astype(f32)
    g["ffb"] = inp["ff_b"].astype(f32)

    g["projT"] = inp["proj_w"].T.copy().astype(f32)
    g["projb"] = inp["proj_b"].astype(f32)
    g["penw"], g["penb"] = inp["pen_w"].astype(f32), inp["pen_b"].astype(f32)

    wr = _resize_weight_mat(H2, H)
    g["WupT"] = np.kron(wr, wr).T.copy().astype(f32)  # [784, 3136]

    import ml_dtypes
    expt = np.exp(inp["rpb_table"].astype(f32))       # [12321, 3]
    rel = np.asarray(inp["rel_index"])                # [L, L] int32 (rel[i, j])
    g["expB"] = np.ascontiguousarray(
        expt[rel.T].transpose(2, 0, 1)).astype(ml_dtypes.bfloat16)
    return g


def _build():
    import contextlib
    import concourse.bass as bass  # noqa: F401
    import concourse.tile as tile
    from concourse import bacc, mybir

    f32, bf16, f32r = mybir.dt.float32, mybir.dt.bfloat16, mybir.dt.float32r
    AF = mybir.ActivationFunctionType
    OP = mybir.AluOpType

    nc = bacc.Bacc("TRN2", target_bir_lowering=False, debug=False,
                   num_devices=NCORES)

    def din(name, shape, dtype=f32):
        return nc.dram_tensor(name, shape, dtype, kind="ExternalInput").ap()

    p1T = din("p1T", [C, L], bf16)
    p1T_own = din("p1T_own", [C, QPC], bf16)
    p2T = din("p2T", [2 * C, L2], bf16)
    WupT = din("WupT", [L2, L], bf16)
    WupT_own = din("WupT_own", [L2, QPC], bf16)
    expB = din("expB", [HEADS, L, QPC], bf16)
    w_projT = din("w_projT", [2 * C, C], bf16)
    v_projb = din("v_projb", [1, C], f32r)
    v_penw, v_penb = din("v_penw", [C]), din("v_penb", [C])
    w_qhT, v_bqh = din("w_qhT", [C, C], bf16), din("v_bqh", [C])
    w_qlT, v_bql = din("w_qlT", [C, C], bf16), din("v_bql", [C])
    w_khT, w_klT = din("w_khT", [C, C], bf16), din("w_klT", [C, C], bf16)
    w_vhT, w_vlT = din("w_vhT", [C, C], bf16), din("w_vlT", [C, C], bf16)
    bq_h_col = din("bq_h_col", [C, 1], bf16)   # scaled q-bias, head-major
    bq_l_col = din("bq_l_col", [C, 1], bf16)
    w_pl1LT, w_pl1RT = din("w_pl1LT", [C, C], bf16), din("w_pl1RT", [C, C], bf16)
    v_pl1b = din("v_pl1b", [C])
    w_pl2T, v_pl2b = din("w_pl2T", [C, C], bf16), din("v_pl2b", [C])
    w_fohT, v_fohb = din("w_fohT", [C, C], f32r), din("v_fohb", [C])
    w_folT, v_folb = din("w_folT", [C, C], f32r), din("v_folb", [C])
    w_g1LT, w_g1RT = din("w_g1LT", [C, C], bf16), din("w_g1RT", [C, C], bf16)
    v_g1b = din("v_g1b", [C])
    w_g2T, v_g2b = din("w_g2T", [C, 1], bf16), din("v_g2b", [1])
    w_ffLT, w_ffPT = din("w_ffLT", [C, C], bf16), din("w_ffPT", [C, C], bf16)
    v_ffb = din("v_ffb", [C])

    outT = nc.dram_tensor("outT", [C, QPC], f32, kind="ExternalOutput").ap()

    def r32(ap):
        return ap.bitcast(f32r)

    with tile.TileContext(nc) as tc:
        with tc.tile_pool(name="const", bufs=1) as const:
            def load_w3(pool, dram, tag, rows=C, dtype=bf16):
                # one batched DMA per weight matrix: partition-tiles land
                # side by side on the free dim
                k = rows // 128
                n = dram.shape[1]
                t = pool.tile([128, k * n], dtype, tag=tag, name=tag)
                nc.sync.dma_start(
                    out=t.rearrange("p (k n) -> p k n", k=k),
                    in_=dram.rearrange("(k p) n -> p k n", p=128))
                return [t[:, i * n:(i + 1) * n] for i in range(k)]

            def load_b3(pool, dram, tag, dtype=f32):
                t = pool.tile([128, CT], dtype, tag=tag, name=tag)
                nc.sync.dma_start(
                    out=t,
                    in_=dram.bitcast(dtype).rearrange("(k p) -> p k", p=128))
                return [t[:, i:i + 1] for i in range(CT)]

            ones_f = const.tile([128, 128], f32, tag="ones_f", name="ones_f")
            nc.vector.memset(ones_f, 1.0)
            ones_b = const.tile([128, 128], bf16, tag="ones_b", name="ones_b")
            nc.vector.memset(ones_b, 1.0)
            eps_t = const.tile([128, 1], f32, tag="eps_t", name="eps_t")
            nc.vector.memset(eps_t, EPS)

            def ln_feature_major(pool, rawpool, ppool, chw, raw, dst_aps):
                """Feature-major LayerNorm core ((x-m)*r over 384 partitions).
                `raw` is a list of CT bf16 [128, chw] APs already produced;
                the normalized result is written directly into `dst_aps`.
                Stats computed in partition-broadcast form via all-ones
                matmuls; stats in f32, broadcast mean and rstd cast to bf16
                so all DVE ops keep uniform dtypes."""
                ps_m = ppool.tile([128, chw], f32, tag="ps_m", name="ps_m")
                for cb in range(CT):
                    nc.tensor.matmul(ps_m, ones_b, raw[cb],
                                     start=(cb == 0), stop=(cb == CT - 1))
                ps_s = ppool.tile([128, chw], f32, tag="ps_s", name="ps_s")
                for cb in range(CT):
                    sq = rawpool.tile([128, chw], bf16, tag="lnsq", name="lnsq")
                    nc.vector.tensor_mul(sq, raw[cb], raw[cb])
                    nc.tensor.matmul(ps_s, ones_b, sq,
                                     start=(cb == 0), stop=(cb == CT - 1))
                m_bf = pool.tile([128, chw], bf16, tag="m_bf", name="m_bf")
                nc.vector.tensor_scalar_mul(m_bf, ps_m, 1.0 / C)
                m2 = pool.tile([128, chw], f32, tag="m2", name="m2")
                nc.vector.tensor_mul(m2, m_bf, m_bf)
                nc.vector.scalar_tensor_tensor(
                    out=m2, in0=ps_s, scalar=1.0 / C, in1=m2,
                    op0=OP.mult, op1=OP.subtract)
                nc.scalar.activation(out=m2, in_=m2, func=AF.Sqrt,
                                     bias=eps_t, scale=1.0)
                r_bc = pool.tile([128, chw], f32, tag="r_bc", name="r_bc")
                nc.vector.reciprocal_approx_fast(out=r_bc, in_=m2)
                r_bf = pool.tile([128, chw], bf16, tag="r_bf", name="r_bf")
                nc.vector.tensor_copy(r_bf, r_bc)
                for cb in range(CT):
                    nc.vector.tensor_sub(raw[cb], raw[cb], m_bf)
                    nc.vector.tensor_mul(dst_aps[cb], raw[cb], r_bf)

            with tc.tile_pool(name="apool", bufs=1) as apool:
                qh = [apool.tile([128, QPC], bf16, tag=f"qh{c}", name=f"qh{c}")
                      for c in range(CT)]
                ql = [apool.tile([128, QPC], bf16, tag=f"ql{c}", name=f"ql{c}")
                      for c in range(CT)]
                oh = [apool.tile([128, QPC], bf16, tag=f"oh{c}", name=f"oh{c}")
                      for c in range(CT)]
                ol = [apool.tile([128, QPC], bf16, tag=f"ol{c}", name=f"ol{c}")
                      for c in range(CT)]

                with tc.tile_pool(name="kvpool", bufs=1) as kvpool:
                    kh = [kvpool.tile([128, L], bf16, tag=f"kh{c}", name=f"kh{c}")
                          for c in range(CT)]
                    kl = [kvpool.tile([128, L], bf16, tag=f"kl{c}", name=f"kl{c}")
                          for c in range(CT)]
                    vh = [kvpool.tile([jn, C], bf16, tag=f"vh{i}", name=f"vh{i}")
                          for i, (_, jn) in enumerate(JTS)]
                    vl = [kvpool.tile([jn, C], bf16, tag=f"vl{i}", name=f"vl{i}")
                          for i, (_, jn) in enumerate(JTS)]
                    kbcol = {}
                    for a in range(2):
                        for h in range(HEADS):
                            kbcol[(a, h)] = kvpool.tile(
                                [128, len(JTS)], f32,
                                tag=f"kbcol{a}{h}", name=f"kbcol{a}{h}")

                    with tc.tile_pool(name="bigpool", bufs=1) as bigpool:
                        p1n = [bigpool.tile([128, L], bf16, tag=f"p1n{c}", name=f"p1n{c}")
                               for c in range(CT)]
                        p2up = [bigpool.tile([128, L], bf16, tag=f"p2up{c}", name=f"p2up{c}")
                                for c in range(CT)]
                        pp = [bigpool.tile([128, L], bf16, tag=f"pp{c}", name=f"pp{c}")
                              for c in range(CT)]
                        xnorm = [bigpool.tile([TT2, C], bf16, tag=f"xnorm{t}", name=f"xnorm{t}")
                                 for t in range(TOK2)]
                        penw3 = load_b3(bigpool, v_penw, "penw")
                        penb3 = load_b3(bigpool, v_penb, "penb")

                        # Phase 1: x = LN_pen_core(p2 @ projT + b), token-major
                        with tc.tile_pool(name="ph1s", bufs=1) as ph1s, \
                             tc.tile_pool(name="ph1t", bufs=3) as ph1, \
                             tc.tile_pool(name="ph1p", bufs=2, space="PSUM") as ph1p:
                            tproj = load_w3(ph1s, w_projT, "projT", rows=2 * C)
                            projb_row = ph1s.tile([1, C], f32r, tag="projb_row", name="projb_row")
                            nc.sync.dma_start(out=projb_row, in_=v_projb)
                            p2s = load_w3(ph1s, p2T, "p2s", rows=2 * C)
                            for tt in range(TOK2):
                                ps = ph1p.tile([TT2, C], f32, tag="ps_x", name="ps_x")
                                sl = slice(tt * TT2, (tt + 1) * TT2)
                                for k in range(6):
                                    nc.tensor.matmul(ps, p2s[k][:, sl], tproj[k],
                                                     start=(k == 0), stop=False)
                                nc.tensor.matmul(ps, r32(ones_f[0:1, 0:TT2]),
                                                 projb_row, start=False, stop=True)
                                st = ph1.tile([TT2, 6], f32, tag="bnst", name="bnst")
                                nc.vector.bn_stats(out=st, in_=ps)
                                mv = ph1.tile([TT2, 2], f32, tag="bnmv", name="bnmv")
                                nc.vector.bn_aggr(out=mv, in_=st)
                                sd = ph1.tile([TT2, 1], f32, tag="sd", name="sd")
                                nc.scalar.activation(out=sd, in_=mv[:, 1:2],
                                                     func=AF.Sqrt,
                                                     bias=eps_t[0:TT2], scale=1.0)
                                rr = ph1.tile([TT2, 1], f32, tag="rr", name="rr")
                                rscr = ph1.tile([TT2, 1], f32, tag="rscr", name="rscr")
                                nc.vector.reciprocal_approx_accurate(
                                    out=rr, in_=sd, scratch=rscr)
                                nmr = ph1.tile([TT2, 1], f32, tag="nmr", name="nmr")
                                nc.vector.scalar_tensor_tensor(
                                    out=nmr, in0=mv[:, 0:1], scalar=-1.0, in1=rr,
                                    op0=OP.mult, op1=OP.mult)
                                nc.scalar.activation(out=xnorm[tt], in_=ps,
                                                     func=AF.Identity,
                                                     bias=nmr, scale=rr)

                        # Merged loop: p2up, p1n, pp chunk by chunk, all SBUF
                        with tc.tile_pool(name="mw", bufs=1) as mw, \
                             tc.tile_pool(name="mt", bufs=1) as mt, \
                             tc.tile_pool(name="mraw", bufs=2) as mraw, \
                             tc.tile_pool(name="mwup", bufs=1) as mwup, \
                             tc.tile_pool(name="mp_up", bufs=2, space="PSUM") as mp_up, \
                             tc.tile_pool(name="mp_st", bufs=1, space="PSUM") as mp_st, \
                             tc.tile_pool(name="mp_pl", bufs=1, space="PSUM") as mp_pl:
                            tl1L = load_w3(mw, w_pl1LT, "pl1LT")
                            tl1R = load_w3(mw, w_pl1RT, "pl1RT")
                            tl2 = load_w3(mw, w_pl2T, "pl2T")
                            bl1 = load_b3(mw, v_pl1b, "pl1b")
                            bl2 = load_b3(mw, v_pl2b, "pl2b")
                            tkh = load_w3(mw, w_khT, "khT")
                            tkl = load_w3(mw, w_klT, "klT")
                            tvh = load_w3(mw, w_vhT, "vhT")
                            tvl = load_w3(mw, w_vlT, "vlT")
                            bqcol = {}
                            for a, dram in ((0, bq_h_col), (1, bq_l_col)):
                                for h in range(HEADS):
                                    t = mw.tile([128, 1], bf16,
                                                tag=f"bqc{a}{h}", name=f"bqc{a}{h}")
                                    nc.sync.dma_start(
                                        out=t, in_=dram[h * 128:(h + 1) * 128, :])
                                    bqcol[(a, h)] = t
                            tqh = load_w3(mw, w_qhT, "qhT")
                            bqh3 = load_b3(mw, v_bqh, "bqh")
                            tql = load_w3(mw, w_qlT, "qlT")
                            bql3 = load_b3(mw, v_bql, "bql")
                            p2upo = [bigpool.tile([128, QC], bf16, tag=f"p2upo{c}", name=f"p2upo{c}")
                                     for c in range(CT)]
                            p1no = [bigpool.tile([128, QC], bf16, tag=f"p1no{c}", name=f"p1no{c}")
                                    for c in range(CT)]
                            v_next = 0

                            def up_raws(wsrc, csl, pstag):
                                # upsample matmuls + pen affine for one chunk
                                wt = mwup.tile([TT2, TOK2 * MC], bf16,
                                               tag="wup_all", name="wup_all")
                                nc.sync.dma_start(
                                    out=wt.rearrange("p (k n) -> p k n", k=TOK2),
                                    in_=wsrc.rearrange("(k p) n -> p k n",
                                                       p=TT2)[:, :, csl])
                                raws = []
                                for cb in range(CT):
                                    ps = mp_up.tile([128, MC], f32, tag=pstag, name=pstag)
                                    for kt in range(TOK2):
                                        nc.tensor.matmul(
                                            ps,
                                            xnorm[kt][:, cb * 128:(cb + 1) * 128],
                                            wt[:, kt * MC:(kt + 1) * MC],
                                            start=(kt == 0), stop=(kt == TOK2 - 1))
                                    r = mraw.tile([128, MC], bf16,
                                                  tag=f"lnraw{cb}", name=f"lnraw{cb}")
                                    nc.vector.tensor_scalar(
                                        out=r, in0=ps, scalar1=penw3[cb],
                                        scalar2=penb3[cb], op0=OP.mult, op1=OP.add)
                                    raws.append(r)
                                return raws

                            def dram_raws(dsrc, csl):
                                # one batched DMA for CT partition-tiles
                                t = mraw.tile([128, CT * MC], bf16,
                                              tag="p1raw", name="p1raw")
                                nc.sync.dma_start(
                                    out=t.rearrange("p (k n) -> p k n", k=CT),
                                    in_=dsrc.rearrange("(k p) n -> p k n",
                                                       p=128)[:, :, csl])
                                return [t[:, i * MC:(i + 1) * MC]
                                        for i in range(CT)]

                            def emit_ph5_chunk(chq):
                                csl = slice(chq * QC, (chq + 1) * QC)
                                raws = up_raws(WupT_own, csl, "ps_up")
                                ln_feature_major(mt, mraw, mp_st, QC, raws,
                                                 p2upo)
                                raws = dram_raws(p1T_own, csl)
                                ln_feature_major(mt, mraw, mp_st, QC, raws,
                                                 p1no)
                                # Q projections for this query chunk
                                for (dst, src, tw, tb) in (
                                        (qh, p1no, tqh, bqh3),
                                        (ql, p2upo, tql, bql3)):
                                    for cb in range(CT):
                                        ps = mp_pl.tile([128, MC], f32,
                                                        tag="ps_k", name="ps_k")
                                        for kt in range(CT):
                                            nc.tensor.matmul(
                                                ps, tw[kt][:, cb * 128:(cb + 1) * 128],
                                                src[kt],
                                                start=(kt == 0),
                                                stop=(kt == CT - 1))
                                        nc.scalar.activation(
                                            out=dst[cb][:, csl], in_=ps,
                                            func=AF.Identity, bias=tb[cb],
                                            scale=1.0)

                            for ch in range(NMC):
                                csl = slice(ch * MC, (ch + 1) * MC)
                                raws = up_raws(WupT, csl, "ps_up")
                                ln_feature_major(mt, mraw, mp_st, MC, raws,
                                                 [p2up[cb][:, csl]
                                                  for cb in range(CT)])
                                raws = dram_raws(p1T, csl)
                                ln_feature_major(mt, mraw, mp_st, MC, raws,
                                                 [p1n[cb][:, csl]
                                                  for cb in range(CT)])
                                gel = []
                                for cb in range(CT):
                                    ps = mp_pl.tile([128, MC], f32, tag="ps_pp", name="ps_pp")
                                    for kt in range(CT):
                                        nc.tensor.matmul(
                                            ps, tl1L[kt][:, cb * 128:(cb + 1) * 128],
                                            p1n[kt][:, csl],
                                            start=(kt == 0), stop=False)
                                    for kt in range(CT):
                                        nc.tensor.matmul(
                                            ps, tl1R[kt][:, cb * 128:(cb + 1) * 128],
                                            p2up[kt][:, csl], start=False,
                                            stop=(kt == CT - 1))
                                    gt = mt.tile([128, MC], bf16, tag=f"gel{cb}", name=f"gel{cb}")
                                    nc.scalar.activation(out=gt, in_=ps,
                                                         func=AF.Gelu,
                                                         bias=bl1[cb], scale=1.0)
                                    gel.append(gt)
                                for cb in range(CT):
                                    ps = mp_pl.tile([128, MC], f32, tag="ps_pp", name="ps_pp")
                                    for kt in range(CT):
                                        nc.tensor.matmul(
                                            ps, tl2[kt][:, cb * 128:(cb + 1) * 128],
                                            gel[kt], start=(kt == 0),
                                            stop=(kt == CT - 1))
                                    nc.vector.tensor_scalar_add(
                                        pp[cb][:, csl], ps, bl2[cb])
                                # inline K projections for this chunk
                                # (kh from p2up, kl from pp)
                                for a, (kk, src, twk) in enumerate(
                                        ((kh, p2up, tkh), (kl, pp, tkl))):
                                    for cb in range(CT):
                                        ps = mp_pl.tile([128, MC], f32,
                                                        tag="ps_k", name="ps_k")
                                        for kt in range(CT):
                                            nc.tensor.matmul(
                                                ps, twk[kt][:, cb * 128:(cb + 1) * 128],
                                                src[kt][:, csl],
                                                start=(kt == 0), stop=(kt == CT - 1))
                                        nc.scalar.activation(
                                            out=kk[cb][:, csl], in_=ps, func=AF.Copy)
                                # V projections + kbcol columns for all
                                # j-tiles fully covered by tokens
                                # [0, (ch+1)*MC): independent PE filler
                                # between the LN dependency chains
                                while v_next < len(JTS) and (
                                        JTS[v_next][0] + JTS[v_next][1]
                                        <= (ch + 1) * MC):
                                    j0, jn = JTS[v_next]
                                    for a, (src, twv, vv, kk) in enumerate(
                                            ((p2up, tvh, vh, kh),
                                             (pp, tvl, vl, kl))):
                                        ps = mp_pl.tile([128, MC], f32,
                                                        tag="ps_v", name="ps_v")
                                        for kt in range(CT):
                                            nc.tensor.matmul(
                                                ps[:jn, 0:C],
                                                src[kt][:, j0:j0 + jn],
                                                twv[kt], start=(kt == 0),
                                                stop=(kt == CT - 1))
                                        nc.scalar.activation(
                                            out=vv[v_next], in_=ps[:jn, 0:C],
                                            func=AF.Copy)
                                        for h in range(HEADS):
                                            ps_c = mp_pl.tile(
                                                [128, 1], f32,
                                                tag="ps_kc", name="ps_kc")
                                            nc.tensor.matmul(
                                                ps_c[:jn], kk[h][:, j0:j0 + jn],
                                                bqcol[(a, h)],
                                                start=True, stop=True)
                                            nc.vector.tensor_copy(
                                                kbcol[(a, h)][:jn,
                                                              v_next:v_next + 1],
                                                ps_c[:jn])
                                    v_next += 1
                                # own-slice recompute interleaved as filler
                                if ch == 4:
                                    emit_ph5_chunk(0)
                                elif ch == 6:
                                    emit_ph5_chunk(1)



                    # Attention (bigpool freed)
                    with tc.tile_pool(name="atw", bufs=1) as atw:
                        tfoh = load_w3(atw, w_fohT, "fohT", dtype=f32r)
                        bfoh = load_b3(atw, v_fohb, "fohb")
                        tfol = load_w3(atw, w_folT, "folT", dtype=f32r)
                        bfol = load_b3(atw, v_folb, "folb")

                        with tc.tile_pool(name="at", bufs=3) as at, \
                             tc.tile_pool(name="atb", bufs=2) as atb, \
                             tc.tile_pool(name="ato", bufs=1) as ato, \
                             tc.tile_pool(name="atps", bufs=2, space="PSUM") as atps, \
                             tc.tile_pool(name="atpo", bufs=2, space="PSUM") as atpo, \
                             tc.tile_pool(name="atpd", bufs=2, space="PSUM") as atpd, \
                             tc.tile_pool(name="atpp", bufs=2, space="PSUM") as atpp:
                            for qc in range(NQC):
                                qsl = slice(qc * QC, (qc + 1) * QC)
                                onorm = {}
                                for h in range(HEADS):
                                    ps_o = [atpo.tile([128, QC], f32, tag="ps_o", name="ps_o")
                                            for _ in range(2)]
                                    ps_d = [atpd.tile([128, QC], f32, tag="ps_d", name="ps_d")
                                            for _ in range(2)]
                                    for g0, g1 in EBG:
                                        nt = g1 - g0
                                        ebg = atb.tile([128, 5 * QC], bf16,
                                                       tag="ebg", name="ebg")
                                        if JTS[g1 - 1][1] == 128:
                                            nc.sync.dma_start(
                                                out=ebg.rearrange(
                                                    "p (t n) -> p t n",
                                                    t=5)[:, :nt, :],
                                                in_=expB[
                                                    h,
                                                    JTS[g0][0]:JTS[g1 - 1][0] + 128,
                                                    qsl].rearrange(
                                                    "(t p) n -> p t n", p=128))
                                        else:
                                            nc.sync.dma_start(
                                                out=ebg[:JTS[g0][1], 0:QC],
                                                in_=expB[h, JTS[g0][0]:L, qsl])
                                        for i in range(g0, g1):
                                            j0, jn = JTS[i]
                                            eb = ebg[:jn, (i - g0) * QC:
                                                     (i - g0 + 1) * QC]
                                            for a, (kk, qq, vv) in enumerate(
                                                    ((kh, qh, vh), (kl, ql, vl))):
                                                ps_s = atps.tile([jn, QC], f32,
                                                                 tag="ps_s", name="ps_s")
                                                nc.tensor.matmul(
                                                    ps_s, kk[h][:, j0:j0 + jn],
                                                    qq[h][:, qsl],
                                                    start=True, stop=True)
                                                aa = at.tile([jn, QC], bf16, tag="aa", name="aa")
                                                nc.scalar.activation(
                                                    out=aa, in_=ps_s, func=AF.Exp,
                                                    bias=kbcol[(a, h)][:jn, i:i + 1],
                                                    scale=1.0)
                                                nc.vector.tensor_mul(aa, aa, eb)
                                                nc.tensor.matmul(
                                                    ps_o[a],
                                                    vv[i][:, h * 128:(h + 1) * 128],
                                                    aa, start=(i == 0),
                                                    stop=(i == len(JTS) - 1))
                                                nc.tensor.matmul(
                                                    ps_d[a], ones_b[:jn], aa,
                                                    start=(i == 0),
                                                    stop=(i == len(JTS) - 1))
                                    for a in range(2):
                                        rden = at.tile([128, QC], f32, tag="rden", name="rden")
                                        nc.vector.reciprocal_approx_fast(
                                            out=rden, in_=ps_d[a])
                                        on = ato.tile([128, QC], f32r,
                                                      tag=f"on{a}{h}", name=f"on{a}{h}")
                                        nc.vector.tensor_mul(on, ps_o[a], rden)
                                        onorm[(a, h)] = on
                                for a, (dst, tw, tb) in enumerate(
                                        ((oh, tfoh, bfoh), (ol, tfol, bfol))):
                                    for cb in range(CT):
                                        ps = atpp.tile([128, QC], f32,
                                                       tag="ps_fo", name="ps_fo")
                                        for h in range(HEADS):
                                            nc.tensor.matmul(
                                                ps, tw[h][:, cb * 128:(cb + 1) * 128],
                                                onorm[(a, h)],
                                                start=(h == 0),
                                                stop=(h == HEADS - 1))
                                        nc.vector.tensor_scalar_add(
                                            dst[cb][:, qsl], ps, tb[cb])

                # gate, mix, ff
                with tc.tile_pool(name="ph8w", bufs=1) as ph8w, \
                     tc.tile_pool(name="ph8", bufs=2) as ph8, \
                     tc.tile_pool(name="ph8p", bufs=2, space="PSUM") as ph8p:
                    tg1L = load_w3(ph8w, w_g1LT, "g1LT")
                    tg1R = load_w3(ph8w, w_g1RT, "g1RT")
                    bg1 = load_b3(ph8w, v_g1b, "g1b")
                    tg2 = load_w3(ph8w, w_g2T, "g2T")
                    g2b_t = ph8w.tile([1, 1], f32, tag="g2b_t", name="g2b_t")
                    nc.sync.dma_start(
                        out=g2b_t, in_=v_g2b.rearrange("(a b) -> a b", a=1))
                    tffL = load_w3(ph8w, w_ffLT, "ffLT")
                    tffP = load_w3(ph8w, w_ffPT, "ffPT")
                    bff = load_b3(ph8w, v_ffb, "ffb")
                    p1o = [ph8w.tile([128, QPC], bf16, tag=f"p1o{c}", name=f"p1o{c}")
                           for c in range(CT)]
                    for cb in range(CT):
                        nc.sync.dma_start(
                            out=p1o[cb],
                            in_=p1T_own[cb * 128:(cb + 1) * 128, :])
                    for qc in range(NQC):
                        qsl = slice(qc * QC, (qc + 1) * QC)
                        gel = []
                        for cb in range(CT):
                            ps = ph8p.tile([128, QC], f32, tag="ps_g1", name="ps_g1")
                            for kt in range(CT):
                                nc.tensor.matmul(
                                    ps, tg1L[kt][:, cb * 128:(cb + 1) * 128],
                                    oh[kt][:, qsl],
                                    start=(kt == 0), stop=False)
                            for kt in range(CT):
                                nc.tensor.matmul(
                                    ps, tg1R[kt][:, cb * 128:(cb + 1) * 128],
                                    ol[kt][:, qsl], start=False,
                                    stop=(kt == CT - 1))
                            gt = ph8.tile([128, QC], bf16, tag=f"ggel{cb}", name=f"ggel{cb}")
                            nc.scalar.activation(out=gt, in_=ps, func=AF.Gelu,
                                                 bias=bg1[cb], scale=1.0)
                            gel.append(gt)
                        ps_z = ph8p.tile([1, QC], f32, tag="ps_z", name="ps_z")
                        for kt in range(CT):
                            nc.tensor.matmul(ps_z, tg2[kt], gel[kt],
                                             start=(kt == 0),
                                             stop=(kt == CT - 1))
                        gate = ph8.tile([1, QC], f32r, tag="gate", name="gate")
                        nc.scalar.activation(out=gate, in_=ps_z,
                                             func=AF.Sigmoid,
                                             bias=g2b_t, scale=1.0)
                        ps_gb = ph8p.tile([128, QC], f32, tag="ps_gb", name="ps_gb")
                        nc.tensor.matmul(ps_gb, r32(ones_f[0:1, :]), gate,
                                         start=True, stop=True)
                        gb_bf = ph8.tile([128, QC], bf16, tag="gb_bf", name="gb_bf")
                        nc.vector.tensor_copy(gb_bf, ps_gb)
                        mix = []
                        for cb in range(CT):
                            dd = ph8.tile([128, QC], bf16, tag="dd", name="dd")
                            nc.vector.tensor_sub(dd, oh[cb][:, qsl],
                                                 ol[cb][:, qsl])
                            d2 = ph8.tile([128, QC], bf16, tag="d2", name="d2")
                            nc.vector.tensor_mul(d2, dd, gb_bf)
                            mx = ph8.tile([128, QC], bf16, tag=f"mix{cb}", name=f"mix{cb}")
                            nc.vector.tensor_add(mx, d2, ol[cb][:, qsl])
                            mix.append(mx)
                        for cb in range(CT):
                            ps = ph8p.tile([128, QC], f32, tag="ps_ff", name="ps_ff")
                            for kt in range(CT):
                                nc.tensor.matmul(
                                    ps, tffL[kt][:, cb * 128:(cb + 1) * 128],
                                    mix[kt], start=(kt == 0), stop=False)
                            for kt in range(CT):
                                nc.tensor.matmul(
                                    ps, tffP[kt][:, cb * 128:(cb + 1) * 128],
                                    p1o[kt][:, qsl], start=False,
                                    stop=(kt == CT - 1))
                            res = ph8.tile([128, QC], f32, tag="res", name="res")
                            nc.vector.tensor_scalar_add(res, ps, bff[cb])
                            nc.sync.dma_start(
                                out=outT[cb * 128:(cb + 1) * 128, qsl],
                                in_=res)

    nc.compile()
    return nc


def _prepare(inputs):
    """Host prep + input sharding. Returns (nc, in_maps)."""
    global _COMPILED
    import ml_dtypes
    bf = ml_dtypes.bfloat16
    inp = {k: np.asarray(v) for k, v in inputs.items()}
    g = _host_prep(inp)

    if _COMPILED is None:
        _COMPILED = _build()
    nc = _COMPILED

    p1 = inp["p1"].astype(np.float32)
    p2 = inp["p2"].astype(np.float32)

    # per-head scaled q-biases for the kb bias row (column layout)
    bq_h_col = g["bqh"].reshape(C, 1).astype(bf)
    bq_l_col = g["bql"].reshape(C, 1).astype(bf)

    shared = {
        "WupT": g["WupT"].astype(bf),
        "w_projT": g["projT"].astype(bf),
        "v_projb": g["projb"].reshape(1, C),
        "v_penw": g["penw"], "v_penb": g["penb"],
        "w_qhT": g["wqhT"].astype(bf), "v_bqh": g["bqh"],
        "w_qlT": g["wqlT"].astype(bf), "v_bql": g["bql"],
        "w_khT": g["wkhT"].astype(bf), "w_klT": g["wklT"].astype(bf),
        "w_vhT": g["wvhT"].astype(bf), "w_vlT": g["wvlT"].astype(bf),
        "bq_h_col": bq_h_col, "bq_l_col": bq_l_col,
        "w_pl1LT": g["pl1LT"].astype(bf), "w_pl1RT": g["pl1RT"].astype(bf),
        "v_pl1b": g["pl1b"],
        "w_pl2T": g["pl2T"].astype(bf), "v_pl2b": g["pl2b"],
        "w_fohT": g["fohT"], "v_fohb": g["fohb"],
        "w_folT": g["folT"], "v_folb": g["folb"],
        "w_g1LT": g["g1LT"].astype(bf), "w_g1RT": g["g1RT"].astype(bf),
        "v_g1b": g["g1b"],
        "w_g2T": g["g2T"].astype(bf), "v_g2b": g["g2b"],
        "w_ffLT": g["ffLT"].astype(bf), "w_ffPT": g["ffPT"].astype(bf),
        "v_ffb": g["ffb"],
    }
    shared = {k: np.ascontiguousarray(v) for k, v in shared.items()}

    in_maps = []
    for core in range(NCORES):
        b, qi = divmod(core, 4)
        q0 = qi * QPC
        m = dict(shared)
        m["p1T"] = np.ascontiguousarray(p1[b].T.astype(bf))
        m["p1T_own"] = np.ascontiguousarray(p1[b, q0:q0 + QPC, :].T.astype(bf))
        m["p2T"] = np.ascontiguousarray(p2[b].T.astype(bf))
        m["WupT_own"] = np.ascontiguousarray(g["WupT"][:, q0:q0 + QPC].astype(bf))
        m["expB"] = np.ascontiguousarray(g["expB"][:, :, q0:q0 + QPC])
        in_maps.append(m)

    return nc, in_maps


def _run(nc, in_maps):
    from concourse.bass_utils import run_bass_kernel_spmd
    res = run_bass_kernel_spmd(nc, in_maps, core_ids=list(range(NCORES)))
    out = np.zeros((B, L, C), np.float32)
    for core in range(NCORES):
        b, qi = divmod(core, 4)
        q0 = qi * QPC
        out[b, q0:q0 + QPC, :] = res.results[core]["outT"].T
    return out


def kernel(**inputs):
    nc, in_maps = _prepare(inputs)
    return _run(nc, in_maps)


# revision 62
# speedup vs baseline: 1.0110x; 1.0110x over previous
"""Trainium2 Bass kernel for nn_CrossfusionBidirectional.

Sharding: 8 cores = (batch b in {0,1}) x (query-row quarter qi in {0..3}).
Each core computes output rows [qi*784, (qi+1)*784) of batch b with zero
cross-core communication; the host concatenates the 8 slices.

v2: bf16 dataflow. All weights and SBUF activations are bf16 (PSUM stays
f32, LayerNorm statistics in f32); the pre-attention stages run as one
merged per-chunk pipeline with p2up/pp kept in SBUF (no DRAM roundtrip).
Attention is computed transposed (S^T[j, q]) with multiplicative rel-pos
bias exp(s + kb) * exp(B); softmax denominators come from an all-ones
matmul whose output is already broadcast across partitions. LayerNorm
affine params and gammas are folded into downstream weights on the host;
K-projection biases drop out via softmax shift invariance; Q-projection
biases enter through the per-key exp bias column; V-projection biases fold
into the output-projection bias because softmax rows sum to one.
"""

import numpy as np

B, L, C, HEADS = 2, 3136, 384, 3
H, H2 = 56, 28
L2 = L // 4
HD = C // HEADS
EPS = 1e-5
NCORES = 8
QPC = L // 4          # 784 query rows per core
CT = C // 128         # 3 feature tiles
NMC, MC = 8, 392      # merged-loop chunking of full L
NQC, QC = 2, 392      # per-core query chunking
TOK2, TT2 = 7, 112    # low-res token tiling (784 = 7*112)
JTS = [(i * 128, 128) for i in range(24)] + [(3072, 64)]   # key tiles
EBG = [(0, 5), (5, 10), (10, 15), (15, 20), (20, 24), (24, 25)]  # eb DMA groups

_COMPILED = None


def _resize_weight_mat(n_in, n_out):
    # jax.image.resize 'linear' half-pixel: triangle kernel, normalized
    scale = n_out / n_in
    sample_f = (np.arange(n_out) + 0.5) / scale - 0.5
    w = 1.0 - np.abs(sample_f[:, None] - np.arange(n_in)[None, :])
    w = np.clip(w, 0.0, 1.0)
    w = w / w.sum(axis=1, keepdims=True)
    return w.astype(np.float32)


def _host_prep(inp):
    f32 = np.float32
    g = {}
    scale = f32(HD ** -0.5)
    n1w, n1b = inp["n1_w"].astype(f32), inp["n1_b"].astype(f32)
    n2w, n2b = inp["n2_w"].astype(f32), inp["n2_b"].astype(f32)

    def fold_in(w, b, lnw, lnb):
        return (w * lnw[None, :]).astype(f32), (b + w @ lnb).astype(f32)

    wqh, bqh = fold_in(inp["wqh_w"], inp["wqh_b"], n2w, n2b)
    wkh, _ = fold_in(inp["wkh_w"], inp["wkh_b"], n1w, n1b)
    wvh, bvh = fold_in(inp["wvh_w"], inp["wvh_b"], n1w, n1b)
    wql, bql = fold_in(inp["wql_w"], inp["wql_b"], n1w, n1b)
    wkl = inp["wkl_w"].astype(f32)
    wvl, bvl = inp["wvl_w"].astype(f32), inp["wvl_b"].astype(f32)

    g["wqhT"], g["bqh"] = (wqh.T * scale).copy(), bqh * scale
    g["wqlT"], g["bql"] = (wql.T * scale).copy(), bql * scale
    g["wkhT"], g["wklT"] = wkh.T.copy(), wkl.T.copy()
    g["wvhT"], g["wvlT"] = wvh.T.copy(), wvl.T.copy()

    pl1L, pl1R = inp["pl1_w"][:, :C], inp["pl1_w"][:, C:]
    pl1Lw, _ = fold_in(pl1L, np.zeros(C, f32), n2w, n2b)
    pl1Rw, _ = fold_in(pl1R, np.zeros(C, f32), n1w, n1b)
    g["pl1LT"], g["pl1RT"] = pl1Lw.T.copy(), pl1Rw.T.copy()
    g["pl1b"] = (inp["pl1_b"] + pl1L @ n2b + pl1R @ n1b).astype(f32)
    g["pl2T"], g["pl2b"] = inp["pl2_w"].T.copy(), inp["pl2_b"].astype(f32)

    gh, gl = f32(inp["gamma_h"][0]), f32(inp["gamma_l"][0])
    g["fohT"] = (inp["foh_w"].T * gh).astype(f32)
    g["fohb"] = ((inp["foh_b"] + inp["foh_w"] @ bvh) * gh).astype(f32)
    g["folT"] = (inp["fol_w"].T * gl).astype(f32)
    g["folb"] = ((inp["fol_b"] + inp["fol_w"] @ bvl) * gl).astype(f32)

    g["g1LT"] = inp["g1_w"][:, :C].T.copy().astype(f32)
    g["g1RT"] = inp["g1_w"][:, C:].T.copy().astype(f32)
    g["g1b"] = inp["g1_b"].astype(f32)
    g["g2T"] = inp["g2_w"].T.copy().astype(f32)   # [384, 1]
    g["g2b"] = inp["g2_b"].astype(f32)            # [1]

    ffL, ffR = inp["ff_w"][:, :C], inp["ff_w"][:, C:]
    g["ffLT"] = ffL.T.copy().astype(f32)
    g["ffPT"] = (ffL + ffR).T.copy().astype(f32)
    g["ffb"] = inp["ff_b"].astype(f32)

    g["projT"] = inp["proj_w"].T.copy().astype(f32)
    g["projb"] = inp["proj_b"].astype(f32)
    g["penw"], g["penb"] = inp["pen_w"].astype(f32), inp["pen_b"].astype(f32)

    wr = _resize_weight_mat(H2, H)
    g["WupT"] = np.kron(wr, wr).T.copy().astype(f32)  # [784, 3136]

    import ml_dtypes
    expt = np.exp(inp["rpb_table"].astype(f32))       # [12321, 3]
    rel = np.asarray(inp["rel_index"])                # [L, L] int32 (rel[i, j])
    g["expB"] = np.ascontiguousarray(
        expt[rel.T].transpose(2, 0, 1)).astype(ml_dtypes.bfloat16)
    return g


def _build():
    import contextlib
    import concourse.bass as bass  # noqa: F401
    import concourse.tile as tile
    from concourse import bacc, mybir

    f32, bf16, f32r = mybir.dt.float32, mybir.dt.bfloat16, mybir.dt.float32r
    AF = mybir.ActivationFunctionType
    OP = mybir.AluOpType

    nc = bacc.Bacc("TRN2", target_bir_lowering=False, debug=False,
                   num_devices=NCORES)

    def din(name, shape, dtype=f32):
        return nc.dram_tensor(name, shape, dtype, kind="ExternalInput").ap()

    p1T = din("p1T", [C, L], bf16)
    p1T_own = din("p1T_own", [C, QPC], bf16)
    p2T = din("p2T", [2 * C, L2], bf16)
    WupT = din("WupT", [L2, L], bf16)
    WupT_own = din("WupT_own", [L2, QPC], bf16)
    expB = din("expB", [HEADS, L, QPC], bf16)
    w_projT = din("w_projT", [2 * C, C], bf16)
    v_projb = din("v_projb", [1, C], f32r)
    v_penw, v_penb = din("v_penw", [C]), din("v_penb", [C])
    w_qhT, v_bqh = din("w_qhT", [C, C], bf16), din("v_bqh", [C])
    w_qlT, v_bql = din("w_qlT", [C, C], bf16), din("v_bql", [C])
    w_khT, w_klT = din("w_khT", [C, C], bf16), din("w_klT", [C, C], bf16)
    w_vhT, w_vlT = din("w_vhT", [C, C], bf16), din("w_vlT", [C, C], bf16)
    bq_h_col = din("bq_h_col", [C, 1], bf16)   # scaled q-bias, head-major
    bq_l_col = din("bq_l_col", [C, 1], bf16)
    w_pl1LT, w_pl1RT = din("w_pl1LT", [C, C], bf16), din("w_pl1RT", [C, C], bf16)
    v_pl1b = din("v_pl1b", [C])
    w_pl2T, v_pl2b = din("w_pl2T", [C, C], bf16), din("v_pl2b", [C])
    w_fohT, v_fohb = din("w_fohT", [C, C], f32r), din("v_fohb", [C])
    w_folT, v_folb = din("w_folT", [C, C], f32r), din("v_folb", [C])
    w_g1LT, w_g1RT = din("w_g1LT", [C, C], bf16), din("w_g1RT", [C, C], bf16)
    v_g1b = din("v_g1b", [C])
    w_g2T, v_g2b = din("w_g2T", [C, 1], bf16), din("v_g2b", [1])
    w_ffLT, w_ffPT = din("w_ffLT", [C, C], bf16), din("w_ffPT", [C, C], bf16)
    v_ffb = din("v_ffb", [C])

    outT = nc.dram_tensor("outT", [C, QPC], f32, kind="ExternalOutput").ap()

    def r32(ap):
        return ap.bitcast(f32r)

    with tile.TileContext(nc) as tc:
        with tc.tile_pool(name="const", bufs=1) as const:
            def load_w3(pool, dram, tag, rows=C, dtype=bf16):
                # one batched DMA per weight matrix: partition-tiles land
                # side by side on the free dim
                k = rows // 128
                n = dram.shape[1]
                t = pool.tile([128, k * n], dtype, tag=tag, name=tag)
                nc.sync.dma_start(
                    out=t.rearrange("p (k n) -> p k n", k=k),
                    in_=dram.rearrange("(k p) n -> p k n", p=128))
                return [t[:, i * n:(i + 1) * n] for i in range(k)]

            def load_b3(pool, dram, tag, dtype=f32):
                t = pool.tile([128, CT], dtype, tag=tag, name=tag)
                nc.sync.dma_start(
                    out=t,
                    in_=dram.bitcast(dtype).rearrange("(k p) -> p k", p=128))
                return [t[:, i:i + 1] for i in range(CT)]

            ones_f = const.tile([128, 128], f32, tag="ones_f", name="ones_f")
            nc.vector.memset(ones_f, 1.0)
            ones_b = const.tile([128, 128], bf16, tag="ones_b", name="ones_b")
            nc.vector.memset(ones_b, 1.0)
            eps_t = const.tile([128, 1], f32, tag="eps_t", name="eps_t")
            nc.vector.memset(eps_t, EPS)

            def ln_feature_major(pool, rawpool, ppool, chw, raw, dst_aps):
                """Feature-major LayerNorm core ((x-m)*r over 384 partitions).
                `raw` is a list of CT bf16 [128, chw] APs already produced;
                the normalized result is written directly into `dst_aps`.
                Stats computed in partition-broadcast form via all-ones
                matmuls; stats in f32, broadcast mean and rstd cast to bf16
                so all DVE ops keep uniform dtypes."""
                ps_m = ppool.tile([128, chw], f32, tag="ps_m", name="ps_m")
                for cb in range(CT):
                    nc.tensor.matmul(ps_m, ones_b, raw[cb],
                                     start=(cb == 0), stop=(cb == CT - 1))
                ps_s = ppool.tile([128, chw], f32, tag="ps_s", name="ps_s")
                for cb in range(CT):
                    sq = rawpool.tile([128, chw], bf16, tag="lnsq", name="lnsq")
                    nc.vector.tensor_mul(sq, raw[cb], raw[cb])
                    nc.tensor.matmul(ps_s, ones_b, sq,
                                     start=(cb == 0), stop=(cb == CT - 1))
                m_bf = pool.tile([128, chw], bf16, tag="m_bf", name="m_bf")
                nc.vector.tensor_scalar_mul(m_bf, ps_m, 1.0 / C)
                m2 = pool.tile([128, chw], f32, tag="m2", name="m2")
                nc.vector.tensor_mul(m2, m_bf, m_bf)
                nc.vector.scalar_tensor_tensor(
                    out=m2, in0=ps_s, scalar=1.0 / C, in1=m2,
                    op0=OP.mult, op1=OP.subtract)
                nc.scalar.activation(out=m2, in_=m2, func=AF.Sqrt,
                                     bias=eps_t, scale=1.0)
                r_bc = pool.tile([128, chw], f32, tag="r_bc", name="r_bc")
                nc.vector.reciprocal_approx_fast(out=r_bc, in_=m2)
                r_bf = pool.tile([128, chw], bf16, tag="r_bf", name="r_bf")
                nc.vector.tensor_copy(r_bf, r_bc)
                for cb in range(CT):
                    nc.vector.tensor_sub(raw[cb], raw[cb], m_bf)
                    nc.vector.tensor_mul(dst_aps[cb], raw[cb], r_bf)

            with tc.tile_pool(name="apool", bufs=1) as apool:
                qh = [apool.tile([128, QPC], bf16, tag=f"qh{c}", name=f"qh{c}")
                      for c in range(CT)]
                ql = [apool.tile([128, QPC], bf16, tag=f"ql{c}", name=f"ql{c}")
                      for c in range(CT)]
                oh = [apool.tile([128, QPC], bf16, tag=f"oh{c}", name=f"oh{c}")
                      for c in range(CT)]
                ol = [apool.tile([128, QPC], bf16, tag=f"ol{c}", name=f"ol{c}")
                      for c in range(CT)]

                with tc.tile_pool(name="kvpool", bufs=1) as kvpool:
                    kh = [kvpool.tile([128, L], bf16, tag=f"kh{c}", name=f"kh{c}")
                          for c in range(CT)]
                    kl = [kvpool.tile([128, L], bf16, tag=f"kl{c}", name=f"kl{c}")
                          for c in range(CT)]
                    vh = [kvpool.tile([jn, C], bf16, tag=f"vh{i}", name=f"vh{i}")
                          for i, (_, jn) in enumerate(JTS)]
                    vl = [kvpool.tile([jn, C], bf16, tag=f"vl{i}", name=f"vl{i}")
                          for i, (_, jn) in enumerate(JTS)]
                    kbcol = {}
                    for a in range(2):
                        for h in range(HEADS):
                            kbcol[(a, h)] = kvpool.tile(
                                [128, len(JTS)], f32,
                                tag=f"kbcol{a}{h}", name=f"kbcol{a}{h}")

                    with tc.tile_pool(name="bigpool", bufs=1) as bigpool:
                        p1n = [bigpool.tile([128, L], bf16, tag=f"p1n{c}", name=f"p1n{c}")
                               for c in range(CT)]
                        p2up = [bigpool.tile([128, L], bf16, tag=f"p2up{c}", name=f"p2up{c}")
                                for c in range(CT)]
                        pp = [bigpool.tile([128, L], bf16, tag=f"pp{c}", name=f"pp{c}")
                              for c in range(CT)]
                        xnorm = [bigpool.tile([TT2, C], bf16, tag=f"xnorm{t}", name=f"xnorm{t}")
                                 for t in range(TOK2)]
                        penw3 = load_b3(bigpool, v_penw, "penw")
                        penb3 = load_b3(bigpool, v_penb, "penb")

                        # Phase 1: x = LN_pen_core(p2 @ projT + b), token-major
                        with tc.tile_pool(name="ph1s", bufs=1) as ph1s, \
                             tc.tile_pool(name="ph1t", bufs=3) as ph1, \
                             tc.tile_pool(name="ph1p", bufs=2, space="PSUM") as ph1p:
                            tproj = load_w3(ph1s, w_projT, "projT", rows=2 * C)
                            projb_row = ph1s.tile([1, C], f32r, tag="projb_row", name="projb_row")
                            nc.sync.dma_start(out=projb_row, in_=v_projb)
                            p2s = load_w3(ph1s, p2T, "p2s", rows=2 * C)
                            for tt in range(TOK2):
                                ps = ph1p.tile([TT2, C], f32, tag="ps_x", name="ps_x")
                                sl = slice(tt * TT2, (tt + 1) * TT2)
                                for k in range(6):
                                    nc.tensor.matmul(ps, p2s[k][:, sl], tproj[k],
                                                     start=(k == 0), stop=False)
                                nc.tensor.matmul(ps, r32(ones_f[0:1, 0:TT2]),
                                                 projb_row, start=False, stop=True)
                                st = ph1.tile([TT2, 6], f32, tag="bnst", name="bnst")
                                nc.vector.bn_stats(out=st, in_=ps)
                                mv = ph1.tile([TT2, 2], f32, tag="bnmv", name="bnmv")
                                nc.vector.bn_aggr(out=mv, in_=st)
                                sd = ph1.tile([TT2, 1], f32, tag="sd", name="sd")
                                nc.scalar.activation(out=sd, in_=mv[:, 1:2],
                                                     func=AF.Sqrt,
                                                     bias=eps_t[0:TT2], scale=1.0)
                                rr = ph1.tile([TT2, 1], f32, tag="rr", name="rr")
                                rscr = ph1.tile([TT2, 1], f32, tag="rscr", name="rscr")
                                nc.vector.reciprocal_approx_accurate(
                                    out=rr, in_=sd, scratch=rscr)
                                nmr = ph1.tile([TT2, 1], f32, tag="nmr", name="nmr")
                                nc.vector.scalar_tensor_tensor(
                                    out=nmr, in0=mv[:, 0:1], scalar=-1.0, in1=rr,
                                    op0=OP.mult, op1=OP.mult)
                                nc.scalar.activation(out=xnorm[tt], in_=ps,
                                                     func=AF.Identity,
                                                     bias=nmr, scale=rr)

                        # Merged loop: p2up, p1n, pp chunk by chunk, all SBUF
                        with tc.tile_pool(name="mw", bufs=1) as mw, \
                             tc.tile_pool(name="mt", bufs=1) as mt, \
                             tc.tile_pool(name="mraw", bufs=2) as mraw, \
                             tc.tile_pool(name="mwup", bufs=1) as mwup, \
                             tc.tile_pool(name="mp_up", bufs=2, space="PSUM") as mp_up, \
                             tc.tile_pool(name="mp_st", bufs=1, space="PSUM") as mp_st, \
                             tc.tile_pool(name="mp_pl", bufs=1, space="PSUM") as mp_pl:
                            tl1L = load_w3(mw, w_pl1LT, "pl1LT")
                            tl1R = load_w3(mw, w_pl1RT, "pl1RT")
                            tl2 = load_w3(mw, w_pl2T, "pl2T")
                            bl1 = load_b3(mw, v_pl1b, "pl1b")
                            bl2 = load_b3(mw, v_pl2b, "pl2b")
                            tkh = load_w3(mw, w_khT, "khT")
                            tkl = load_w3(mw, w_klT, "klT")
                            tvh = load_w3(mw, w_vhT, "vhT")
                            tvl = load_w3(mw, w_vlT, "vlT")
                            bqcol = {}
                            for a, dram in ((0, bq_h_col), (1, bq_l_col)):
                                for h in range(HEADS):
                                    t = mw.tile([128, 1], bf16,
                                                tag=f"bqc{a}{h}", name=f"bqc{a}{h}")
                                    nc.sync.dma_start(
                                        out=t, in_=dram[h * 128:(h + 1) * 128, :])
                                    bqcol[(a, h)] = t
                            tqh = load_w3(mw, w_qhT, "qhT")
                            bqh3 = load_b3(mw, v_bqh, "bqh")
                            tql = load_w3(mw, w_qlT, "qlT")
                            bql3 = load_b3(mw, v_bql, "bql")
                            p2upo = [bigpool.tile([128, QC], bf16, tag=f"p2upo{c}", name=f"p2upo{c}")
                                     for c in range(CT)]
                            p1no = [bigpool.tile([128, QC], bf16, tag=f"p1no{c}", name=f"p1no{c}")
                                    for c in range(CT)]
                            v_next = 0

                            def up_raws(wsrc, csl, pstag):
                                # upsample matmuls + pen affine for one chunk
                                wt = mwup.tile([TT2, TOK2 * MC], bf16,
                                               tag="wup_all", name="wup_all")
                                nc.sync.dma_start(
                                    out=wt.rearrange("p (k n) -> p k n", k=TOK2),
                                    in_=wsrc.rearrange("(k p) n -> p k n",
                                                       p=TT2)[:, :, csl])
                                raws = []
                                for cb in range(CT):
                                    ps = mp_up.tile([128, MC], f32, tag=pstag, name=pstag)
                                    for kt in range(TOK2):
                                        nc.tensor.matmul(
                                            ps,
                                            xnorm[kt][:, cb * 128:(cb + 1) * 128],
                                            wt[:, kt * MC:(kt + 1) * MC],
                                            start=(kt == 0), stop=(kt == TOK2 - 1))
                                    r = mraw.tile([128, MC], bf16,
                                                  tag=f"lnraw{cb}", name=f"lnraw{cb}")
                                    nc.vector.tensor_scalar(
                                        out=r, in0=ps, scalar1=penw3[cb],
                                        scalar2=penb3[cb], op0=OP.mult, op1=OP.add)
                                    raws.append(r)
                                return raws

                            def dram_raws(dsrc, csl):
                                # one batched DMA for CT partition-tiles
                                t = mraw.tile([128, CT * MC], bf16,
                                              tag="p1raw", name="p1raw")
                                nc.sync.dma_start(
                                    out=t.rearrange("p (k n) -> p k n", k=CT),
                                    in_=dsrc.rearrange("(k p) n -> p k n",
                                                       p=128)[:, :, csl])
                                return [t[:, i * MC:(i + 1) * MC]
                                        for i in range(CT)]

                            def emit_ph5_chunk(chq):
                                csl = slice(chq * QC, (chq + 1) * QC)
                                raws = up_raws(WupT_own, csl, "ps_up")
                                ln_feature_major(mt, mraw, mp_st, QC, raws,
                                                 p2upo)
                                raws = dram_raws(p1T_own, csl)
                                ln_feature_major(mt, mraw, mp_st, QC, raws,
                                                 p1no)
                                # Q projections for this query chunk
                                for (dst, src, tw, tb) in (
                                        (qh, p1no, tqh, bqh3),
                                        (ql, p2upo, tql, bql3)):
                                    for cb in range(CT):
                                        ps = mp_pl.tile([128, MC], f32,
                                                        tag="ps_k", name="ps_k")
                                        for kt in range(CT):
                                            nc.tensor.matmul(
                                                ps, tw[kt][:, cb * 128:(cb + 1) * 128],
                                                src[kt],
                                                start=(kt == 0),
                                                stop=(kt == CT - 1))
                                        nc.scalar.activation(
                                            out=dst[cb][:, csl], in_=ps,
                                            func=AF.Identity, bias=tb[cb],
                                            scale=1.0)

                            for ch in range(NMC):
                                csl = slice(ch * MC, (ch + 1) * MC)
                                raws = up_raws(WupT, csl, "ps_up")
                                ln_feature_major(mt, mraw, mp_st, MC, raws,
                                                 [p2up[cb][:, csl]
                                                  for cb in range(CT)])
                                raws = dram_raws(p1T, csl)
                                ln_feature_major(mt, mraw, mp_st, MC, raws,
                                                 [p1n[cb][:, csl]
                                                  for cb in range(CT)])
                                gel = []
                                for cb in range(CT):
                                    ps = mp_pl.tile([128, MC], f32, tag="ps_pp", name="ps_pp")
                                    for kt in range(CT):
                                        nc.tensor.matmul(
                                            ps, tl1L[kt][:, cb * 128:(cb + 1) * 128],
                                            p1n[kt][:, csl],
                                            start=(kt == 0), stop=False)
                                    for kt in range(CT):
                                        nc.tensor.matmul(
                                            ps, tl1R[kt][:, cb * 128:(cb + 1) * 128],
                                            p2up[kt][:, csl], start=False,
                                            stop=(kt == CT - 1))
                                    gt = mt.tile([128, MC], bf16, tag=f"gel{cb}", name=f"gel{cb}")
                                    nc.scalar.activation(out=gt, in_=ps,
                                                         func=AF.Gelu,
                                                         bias=bl1[cb], scale=1.0)
                                    gel.append(gt)
                                for cb in range(CT):
                                    ps = mp_pl.tile([128, MC], f32, tag="ps_pp", name="ps_pp")
                                    for kt in range(CT):
                                        nc.tensor.matmul(
                                            ps, tl2[kt][:, cb * 128:(cb + 1) * 128],
                                            gel[kt], start=(kt == 0),
                                            stop=(kt == CT - 1))
                                    nc.vector.tensor_scalar_add(
                                        pp[cb][:, csl], ps, bl2[cb])
                                # inline K projections for this chunk
                                # (kh from p2up, kl from pp)
                                for a, (kk, src, twk) in enumerate(
                                        ((kh, p2up, tkh), (kl, pp, tkl))):
                                    for cb in range(CT):
                                        ps = mp_pl.tile([128, MC], f32,
                                                        tag="ps_k", name="ps_k")
                                        for kt in range(CT):
                                            nc.tensor.matmul(
                                                ps, twk[kt][:, cb * 128:(cb + 1) * 128],
                                                src[kt][:, csl],
                                                start=(kt == 0), stop=(kt == CT - 1))
                                        nc.scalar.activation(
                                            out=kk[cb][:, csl], in_=ps, func=AF.Copy)
                                # V projections + kbcol columns for all
                                # j-tiles fully covered by tokens
                                # [0, (ch+1)*MC): independent PE filler
                                # between the LN dependency chains
                                while v_next < len(JTS) and (
                                        JTS[v_next][0] + JTS[v_next][1]
                                        <= (ch + 1) * MC):
                                    j0, jn = JTS[v_next]
                                    for a, (src, twv, vv, kk) in enumerate(
                                            ((p2up, tvh, vh, kh),
                                             (pp, tvl, vl, kl))):
                                        ps = mp_pl.tile([128, MC], f32,
                                                        tag="ps_v", name="ps_v")
                                        for kt in range(CT):
                                            nc.tensor.matmul(
                                                ps[:jn, 0:C],
                                                src[kt][:, j0:j0 + jn],
                                                twv[kt], start=(kt == 0),
                                                stop=(kt == CT - 1))
                                        nc.scalar.activation(
                                            out=vv[v_next], in_=ps[:jn, 0:C],
                                            func=AF.Copy)
                                        for h in range(HEADS):
                                            ps_c = mp_pl.tile(
                                                [128, 1], f32,
                                                tag="ps_kc", name="ps_kc")
                                            nc.tensor.matmul(
                                                ps_c[:jn], kk[h][:, j0:j0 + jn],
                                                bqcol[(a, h)],
                                                start=True, stop=True)
                                            nc.vector.tensor_copy(
                                                kbcol[(a, h)][:jn,
                                                              v_next:v_next + 1],
                                                ps_c[:jn])
                                    v_next += 1
                                # own-slice recompute interleaved as filler;
                                # chunk 1 feeds only attention qc=1, so it
                                # goes last to pad the merged-loop tail
                                if ch == 4:
                                    emit_ph5_chunk(0)
                                elif ch == NMC - 1:
                                    emit_ph5_chunk(1)



                    # Attention (bigpool freed)
                    with tc.tile_pool(name="atw", bufs=1) as atw:
                        tfoh = load_w3(atw, w_fohT, "fohT", dtype=f32r)
                        bfoh = load_b3(atw, v_fohb, "fohb")
                        tfol = load_w3(atw, w_folT, "folT", dtype=f32r)
                        bfol = load_b3(atw, v_folb, "folb")

                        with tc.tile_pool(name="at", bufs=4) as at, \
                             tc.tile_pool(name="atb", bufs=3) as atb, \
                             tc.tile_pool(name="ato", bufs=1) as ato, \
                             tc.tile_pool(name="atps", bufs=2, space="PSUM") as atps, \
                             tc.tile_pool(name="atpo", bufs=2, space="PSUM") as atpo, \
                             tc.tile_pool(name="atpd", bufs=2, space="PSUM") as atpd, \
                             tc.tile_pool(name="atpp", bufs=2, space="PSUM") as atpp:
                            for qc in range(NQC):
                                qsl = slice(qc * QC, (qc + 1) * QC)
                                onorm = {}
                                for h in range(HEADS):
                                    ps_o = [atpo.tile([128, QC], f32, tag="ps_o", name="ps_o")
                                            for _ in range(2)]
                                    ps_d = [atpd.tile([128, QC], f32, tag="ps_d", name="ps_d")
                                            for _ in range(2)]
                                    for g0, g1 in EBG:
                                        nt = g1 - g0
                                        ebg = atb.tile([128, 5 * QC], bf16,
                                                       tag="ebg", name="ebg")
                                        if JTS[g1 - 1][1] == 128:
                                            nc.sync.dma_start(
                                                out=ebg.rearrange(
                                                    "p (t n) -> p t n",
                                                    t=5)[:, :nt, :],
                                                in_=expB[
                                                    h,
                                                    JTS[g0][0]:JTS[g1 - 1][0] + 128,
                                                    qsl].rearrange(
                                                    "(t p) n -> p t n", p=128))
                                        else:
                                            nc.sync.dma_start(
                                                out=ebg[:JTS[g0][1], 0:QC],
                                                in_=expB[h, JTS[g0][0]:L, qsl])
                                        for i in range(g0, g1):
                                            j0, jn = JTS[i]
                                            eb = ebg[:jn, (i - g0) * QC:
                                                     (i - g0 + 1) * QC]
                                            for a, (kk, qq, vv) in enumerate(
                                                    ((kh, qh, vh), (kl, ql, vl))):
                                                ps_s = atps.tile([jn, QC], f32,
                                                                 tag="ps_s", name="ps_s")
                                                nc.tensor.matmul(
                                                    ps_s, kk[h][:, j0:j0 + jn],
                                                    qq[h][:, qsl],
                                                    start=True, stop=True)
                                                aa = at.tile([jn, QC], bf16, tag="aa", name="aa")
                                                nc.scalar.activation(
                                                    out=aa, in_=ps_s, func=AF.Exp,
                                                    bias=kbcol[(a, h)][:jn, i:i + 1],
                                                    scale=1.0)
                                                nc.vector.tensor_mul(aa, aa, eb)
                                                nc.tensor.matmul(
                                                    ps_o[a],
                                                    vv[i][:, h * 128:(h + 1) * 128],
                                                    aa, start=(i == 0),
                                                    stop=(i == len(JTS) - 1))
                                                nc.tensor.matmul(
                                                    ps_d[a], ones_b[:jn], aa,
                                                    start=(i == 0),
                                                    stop=(i == len(JTS) - 1))
                                    for a in range(2):
                                        rden = at.tile([128, QC], f32, tag="rden", name="rden")
                                        nc.vector.reciprocal_approx_fast(
                                            out=rden, in_=ps_d[a])
                                        on = ato.tile([128, QC], f32r,
                                                      tag=f"on{a}{h}", name=f"on{a}{h}")
                                        nc.vector.tensor_mul(on, ps_o[a], rden)
                                        onorm[(a, h)] = on
                                for a, (dst, tw, tb) in enumerate(
                                        ((oh, tfoh, bfoh), (ol, tfol, bfol))):
                                    for cb in range(CT):
                                        ps = atpp.tile([128, QC], f32,
                                                       tag="ps_fo", name="ps_fo")
                                        for h in range(HEADS):
                                            nc.tensor.matmul(
                                                ps, tw[h][:, cb * 128:(cb + 1) * 128],
                                                onorm[(a, h)],
                                                start=(h == 0),
                                                stop=(h == HEADS - 1))
                                        nc.vector.tensor_scalar_add(
                                            dst[cb][:, qsl], ps, tb[cb])

                # gate, mix, ff
                with tc.tile_pool(name="ph8w", bufs=1) as ph8w, \
                     tc.tile_pool(name="ph8", bufs=2) as ph8, \
                     tc.tile_pool(name="ph8p", bufs=2, space="PSUM") as ph8p:
                    tg1L = load_w3(ph8w, w_g1LT, "g1LT")
                    tg1R = load_w3(ph8w, w_g1RT, "g1RT")
                    bg1 = load_b3(ph8w, v_g1b, "g1b")
                    tg2 = load_w3(ph8w, w_g2T, "g2T")
                    g2b_t = ph8w.tile([1, 1], f32, tag="g2b_t", name="g2b_t")
                    nc.sync.dma_start(
                        out=g2b_t, in_=v_g2b.rearrange("(a b) -> a b", a=1))
                    tffL = load_w3(ph8w, w_ffLT, "ffLT")
                    tffP = load_w3(ph8w, w_ffPT, "ffPT")
                    bff = load_b3(ph8w, v_ffb, "ffb")
                    p1o = [ph8w.tile([128, QPC], bf16, tag=f"p1o{c}", name=f"p1o{c}")
                           for c in range(CT)]
                    for cb in range(CT):
                        nc.sync.dma_start(
                            out=p1o[cb],
                            in_=p1T_own[cb * 128:(cb + 1) * 128, :])
                    for qc in range(NQC):
                        qsl = slice(qc * QC, (qc + 1) * QC)
                        gel = []
                        for cb in range(CT):
                            ps = ph8p.tile([128, QC], f32, tag="ps_g1", name="ps_g1")
                            for kt in range(CT):
                                nc.tensor.matmul(
                                    ps, tg1L[kt][:, cb * 128:(cb + 1) * 128],
                                    oh[kt][:, qsl],
                                    start=(kt == 0), stop=False)
                            for kt in range(CT):
                                nc.tensor.matmul(
                                    ps, tg1R[kt][:, cb * 128:(cb + 1) * 128],
                                    ol[kt][:, qsl], start=False,
                                    stop=(kt == CT - 1))
                            gt = ph8.tile([128, QC], bf16, tag=f"ggel{cb}", name=f"ggel{cb}")
                            nc.scalar.activation(out=gt, in_=ps, func=AF.Gelu,
                                                 bias=bg1[cb], scale=1.0)
                            gel.append(gt)
                        ps_z = ph8p.tile([1, QC], f32, tag="ps_z", name="ps_z")
                        for kt in range(CT):
                            nc.tensor.matmul(ps_z, tg2[kt], gel[kt],
                                             start=(kt == 0),
                                             stop=(kt == CT - 1))
                        gate = ph8.tile([1, QC], f32r, tag="gate", name="gate")
                        nc.scalar.activation(out=gate, in_=ps_z,
                                             func=AF.Sigmoid,
                                             bias=g2b_t, scale=1.0)
                        ps_gb = ph8p.tile([128, QC], f32, tag="ps_gb", name="ps_gb")
                        nc.tensor.matmul(ps_gb, r32(ones_f[0:1, :]), gate,
                                         start=True, stop=True)
                        gb_bf = ph8.tile([128, QC], bf16, tag="gb_bf", name="gb_bf")
                        nc.vector.tensor_copy(gb_bf, ps_gb)
                        mix = []
                        for cb in range(CT):
                            dd = ph8.tile([128, QC], bf16, tag="dd", name="dd")
                            nc.vector.tensor_sub(dd, oh[cb][:, qsl],
                                                 ol[cb][:, qsl])
                            d2 = ph8.tile([128, QC], bf16, tag="d2", name="d2")
                            nc.vector.tensor_mul(d2, dd, gb_bf)
                            mx = ph8.tile([128, QC], bf16, tag=f"mix{cb}", name=f"mix{cb}")
                            nc.vector.tensor_add(mx, d2, ol[cb][:, qsl])
                            mix.append(mx)
                        for cb in range(CT):
                            ps = ph8p.tile([128, QC], f32, tag="ps_ff", name="ps_ff")
                            for kt in range(CT):
                                nc.tensor.matmul(
                                    ps, tffL[kt][:, cb * 128:(cb + 1) * 128],
                                    mix[kt], start=(kt == 0), stop=False)
                            for kt in range(CT):
                                nc.tensor.matmul(
                                    ps, tffP[kt][:, cb * 128:(cb + 1) * 128],
                                    p1o[kt][:, qsl], start=False,
                                    stop=(kt == CT - 1))
                            res = ph8.tile([128, QC], f32, tag="res", name="res")
                            nc.vector.tensor_scalar_add(res, ps, bff[cb])
                            nc.sync.dma_start(
                                out=outT[cb * 128:(cb + 1) * 128, qsl],
                                in_=res)

    nc.compile()
    return nc


def _prepare(inputs):
    """Host prep + input sharding. Returns (nc, in_maps)."""
    global _COMPILED
    import ml_dtypes
    bf = ml_dtypes.bfloat16
    inp = {k: np.asarray(v) for k, v in inputs.items()}
    g = _host_prep(inp)

    if _COMPILED is None:
        _COMPILED = _build()
    nc = _COMPILED

    p1 = inp["p1"].astype(np.float32)
    p2 = inp["p2"].astype(np.float32)

    # per-head scaled q-biases for the kb bias row (column layout)
    bq_h_col = g["bqh"].reshape(C, 1).astype(bf)
    bq_l_col = g["bql"].reshape(C, 1).astype(bf)

    shared = {
        "WupT": g["WupT"].astype(bf),
        "w_projT": g["projT"].astype(bf),
        "v_projb": g["projb"].reshape(1, C),
        "v_penw": g["penw"], "v_penb": g["penb"],
        "w_qhT": g["wqhT"].astype(bf), "v_bqh": g["bqh"],
        "w_qlT": g["wqlT"].astype(bf), "v_bql": g["bql"],
        "w_khT": g["wkhT"].astype(bf), "w_klT": g["wklT"].astype(bf),
        "w_vhT": g["wvhT"].astype(bf), "w_vlT": g["wvlT"].astype(bf),
        "bq_h_col": bq_h_col, "bq_l_col": bq_l_col,
        "w_pl1LT": g["pl1LT"].astype(bf), "w_pl1RT": g["pl1RT"].astype(bf),
        "v_pl1b": g["pl1b"],
        "w_pl2T": g["pl2T"].astype(bf), "v_pl2b": g["pl2b"],
        "w_fohT": g["fohT"], "v_fohb": g["fohb"],
        "w_folT": g["folT"], "v_folb": g["folb"],
        "w_g1LT": g["g1LT"].astype(bf), "w_g1RT": g["g1RT"].astype(bf),
        "v_g1b": g["g1b"],
        "w_g2T": g["g2T"].astype(bf), "v_g2b": g["g2b"],
        "w_ffLT": g["ffLT"].astype(bf), "w_ffPT": g["ffPT"].astype(bf),
        "v_ffb": g["ffb"],
    }
    shared = {k: np.ascontiguousarray(v) for k, v in shared.items()}

    in_maps = []
    for core in range(NCORES):
        b, qi = divmod(core, 4)
        q0 = qi * QPC
        m = dict(shared)
        m["p1T"] = np.ascontiguousarray(p1[b].T.astype(bf))
        m["p1T_own"] = np.ascontiguousarray(p1[b, q0:q0 + QPC, :].T.astype(bf))
        m["p2T"] = np.ascontiguousarray(p2[b].T.astype(bf))
        m["WupT_own"] = np.ascontiguousarray(g["WupT"][:, q0:q0 + QPC].astype(bf))
        m["expB"] = np.ascontiguousarray(g["expB"][:, :, q0:q0 + QPC])
        in_maps.append(m)

    return nc, in_maps


def _run(nc, in_maps):
    from concourse.bass_utils import run_bass_kernel_spmd
    res = run_bass_kernel_spmd(nc, in_maps, core_ids=list(range(NCORES)))
    out = np.zeros((B, L, C), np.float32)
    for core in range(NCORES):
        b, qi = divmod(core, 4)
        q0 = qi * QPC
        out[b, q0:q0 + QPC, :] = res.results[core]["outT"].T
    return out


def kernel(**inputs):
    nc, in_maps = _prepare(inputs)
    return _run(nc, in_maps)


# revision 64
# speedup vs baseline: 1.0252x; 1.0141x over previous
"""Trainium2 Bass kernel for nn_CrossfusionBidirectional.

Sharding: 8 cores = (batch b in {0,1}) x (query-row quarter qi in {0..3}).
Each core computes output rows [qi*784, (qi+1)*784) of batch b with zero
cross-core communication; the host concatenates the 8 slices.

v2: bf16 dataflow. All weights and SBUF activations are bf16 (PSUM stays
f32, LayerNorm statistics in f32); the pre-attention stages run as one
merged per-chunk pipeline with p2up/pp kept in SBUF (no DRAM roundtrip).
Attention is computed transposed (S^T[j, q]) with multiplicative rel-pos
bias exp(s + kb) * exp(B); softmax denominators come from an all-ones
matmul whose output is already broadcast across partitions. LayerNorm
affine params and gammas are folded into downstream weights on the host;
K-projection biases drop out via softmax shift invariance; Q-projection
biases enter through the per-key exp bias column; V-projection biases fold
into the output-projection bias because softmax rows sum to one.
"""

import numpy as np

B, L, C, HEADS = 2, 3136, 384, 3
H, H2 = 56, 28
L2 = L // 4
HD = C // HEADS
EPS = 1e-5
NCORES = 8
QPC = L // 4          # 784 query rows per core
CT = C // 128         # 3 feature tiles
NMC, MC = 8, 392      # merged-loop chunking of full L
NQC, QC = 2, 392      # per-core query chunking
TOK2, TT2 = 7, 112    # low-res token tiling (784 = 7*112)
JTS = [(i * 128, 128) for i in range(24)] + [(3072, 64)]   # key tiles
EBG = [(0, 5), (5, 10), (10, 15), (15, 20), (20, 24), (24, 25)]  # eb DMA groups

_COMPILED = None


def _resize_weight_mat(n_in, n_out):
    # jax.image.resize 'linear' half-pixel: triangle kernel, normalized
    scale = n_out / n_in
    sample_f = (np.arange(n_out) + 0.5) / scale - 0.5
    w = 1.0 - np.abs(sample_f[:, None] - np.arange(n_in)[None, :])
    w = np.clip(w, 0.0, 1.0)
    w = w / w.sum(axis=1, keepdims=True)
    return w.astype(np.float32)


def _host_prep(inp):
    f32 = np.float32
    g = {}
    scale = f32(HD ** -0.5)
    n1w, n1b = inp["n1_w"].astype(f32), inp["n1_b"].astype(f32)
    n2w, n2b = inp["n2_w"].astype(f32), inp["n2_b"].astype(f32)

    def fold_in(w, b, lnw, lnb):
        return (w * lnw[None, :]).astype(f32), (b + w @ lnb).astype(f32)

    wqh, bqh = fold_in(inp["wqh_w"], inp["wqh_b"], n2w, n2b)
    wkh, _ = fold_in(inp["wkh_w"], inp["wkh_b"], n1w, n1b)
    wvh, bvh = fold_in(inp["wvh_w"], inp["wvh_b"], n1w, n1b)
    wql, bql = fold_in(inp["wql_w"], inp["wql_b"], n1w, n1b)
    wkl = inp["wkl_w"].astype(f32)
    wvl, bvl = inp["wvl_w"].astype(f32), inp["wvl_b"].astype(f32)

    g["wqhT"], g["bqh"] = (wqh.T * scale).copy(), bqh * scale
    g["wqlT"], g["bql"] = (wql.T * scale).copy(), bql * scale
    g["wkhT"], g["wklT"] = wkh.T.copy(), wkl.T.copy()
    g["wvhT"], g["wvlT"] = wvh.T.copy(), wvl.T.copy()

    pl1L, pl1R = inp["pl1_w"][:, :C], inp["pl1_w"][:, C:]
    pl1Lw, _ = fold_in(pl1L, np.zeros(C, f32), n2w, n2b)
    pl1Rw, _ = fold_in(pl1R, np.zeros(C, f32), n1w, n1b)
    g["pl1LT"], g["pl1RT"] = pl1Lw.T.copy(), pl1Rw.T.copy()
    g["pl1b"] = (inp["pl1_b"] + pl1L @ n2b + pl1R @ n1b).astype(f32)
    g["pl2T"], g["pl2b"] = inp["pl2_w"].T.copy(), inp["pl2_b"].astype(f32)

    gh, gl = f32(inp["gamma_h"][0]), f32(inp["gamma_l"][0])
    g["fohT"] = (inp["foh_w"].T * gh).astype(f32)
    g["fohb"] = ((inp["foh_b"] + inp["foh_w"] @ bvh) * gh).astype(f32)
    g["folT"] = (inp["fol_w"].T * gl).astype(f32)
    g["folb"] = ((inp["fol_b"] + inp["fol_w"] @ bvl) * gl).astype(f32)

    g["g1LT"] = inp["g1_w"][:, :C].T.copy().astype(f32)
    g["g1RT"] = inp["g1_w"][:, C:].T.copy().astype(f32)
    g["g1b"] = inp["g1_b"].astype(f32)
    g["g2T"] = inp["g2_w"].T.copy().astype(f32)   # [384, 1]
    g["g2b"] = inp["g2_b"].astype(f32)            # [1]

    ffL, ffR = inp["ff_w"][:, :C], inp["ff_w"][:, C:]
    g["ffLT"] = ffL.T.copy().astype(f32)
    g["ffPT"] = (ffL + ffR).T.copy().astype(f32)
    g["ffb"] = inp["ff_b"].astype(f32)

    g["projT"] = inp["proj_w"].T.copy().astype(f32)
    g["projb"] = inp["proj_b"].astype(f32)
    g["penw"], g["penb"] = inp["pen_w"].astype(f32), inp["pen_b"].astype(f32)

    wr = _resize_weight_mat(H2, H)
    g["WupT"] = np.kron(wr, wr).T.copy().astype(f32)  # [784, 3136]

    import ml_dtypes
    expt = np.exp(inp["rpb_table"].astype(f32))       # [12321, 3]
    rel = np.asarray(inp["rel_index"])                # [L, L] int32 (rel[i, j])
    g["expB"] = np.ascontiguousarray(
        expt[rel.T].transpose(2, 0, 1)).astype(ml_dtypes.bfloat16)
    return g


def _build():
    import contextlib
    import concourse.bass as bass  # noqa: F401
    import concourse.tile as tile
    from concourse import bacc, mybir

    f32, bf16, f32r = mybir.dt.float32, mybir.dt.bfloat16, mybir.dt.float32r
    AF = mybir.ActivationFunctionType
    OP = mybir.AluOpType

    nc = bacc.Bacc("TRN2", target_bir_lowering=False, debug=False,
                   num_devices=NCORES)

    def din(name, shape, dtype=f32):
        return nc.dram_tensor(name, shape, dtype, kind="ExternalInput").ap()

    p1T = din("p1T", [C, L], bf16)
    p1T_own = din("p1T_own", [C, QPC], bf16)
    p2T = din("p2T", [2 * C, L2], bf16)
    WupT = din("WupT", [L2, L], bf16)
    WupT_own = din("WupT_own", [L2, QPC], bf16)
    expB = din("expB", [HEADS, L, QPC], bf16)
    w_projT = din("w_projT", [2 * C, C], bf16)
    v_projb = din("v_projb", [1, C], f32r)
    v_penw, v_penb = din("v_penw", [C]), din("v_penb", [C])
    w_qhT, v_bqh = din("w_qhT", [C, C], bf16), din("v_bqh", [C])
    w_qlT, v_bql = din("w_qlT", [C, C], bf16), din("v_bql", [C])
    w_khT, w_klT = din("w_khT", [C, C], bf16), din("w_klT", [C, C], bf16)
    w_vhT, w_vlT = din("w_vhT", [C, C], bf16), din("w_vlT", [C, C], bf16)
    bq_h_col = din("bq_h_col", [C, 1], bf16)   # scaled q-bias, head-major
    bq_l_col = din("bq_l_col", [C, 1], bf16)
    w_pl1LT, w_pl1RT = din("w_pl1LT", [C, C], bf16), din("w_pl1RT", [C, C], bf16)
    v_pl1b = din("v_pl1b", [C])
    w_pl2T, v_pl2b = din("w_pl2T", [C, C], bf16), din("v_pl2b", [C])
    w_fohT, v_fohb = din("w_fohT", [C, C], f32r), din("v_fohb", [C])
    w_folT, v_folb = din("w_folT", [C, C], f32r), din("v_folb", [C])
    w_g1LT, w_g1RT = din("w_g1LT", [C, C], bf16), din("w_g1RT", [C, C], bf16)
    v_g1b = din("v_g1b", [C])
    w_g2T, v_g2b = din("w_g2T", [C, 1], bf16), din("v_g2b", [1])
    w_ffLT, w_ffPT = din("w_ffLT", [C, C], bf16), din("w_ffPT", [C, C], bf16)
    v_ffb = din("v_ffb", [C])

    outT = nc.dram_tensor("outT", [C, QPC], f32, kind="ExternalOutput").ap()

    def r32(ap):
        return ap.bitcast(f32r)

    with tile.TileContext(nc) as tc:
        with tc.tile_pool(name="const", bufs=1) as const:
            def load_w3(pool, dram, tag, rows=C, dtype=bf16):
                # one batched DMA per weight matrix: partition-tiles land
                # side by side on the free dim
                k = rows // 128
                n = dram.shape[1]
                t = pool.tile([128, k * n], dtype, tag=tag, name=tag)
                nc.sync.dma_start(
                    out=t.rearrange("p (k n) -> p k n", k=k),
                    in_=dram.rearrange("(k p) n -> p k n", p=128))
                return [t[:, i * n:(i + 1) * n] for i in range(k)]

            def load_b3(pool, dram, tag, dtype=f32):
                t = pool.tile([128, CT], dtype, tag=tag, name=tag)
                nc.sync.dma_start(
                    out=t,
                    in_=dram.bitcast(dtype).rearrange("(k p) -> p k", p=128))
                return [t[:, i:i + 1] for i in range(CT)]

            ones_f = const.tile([128, 128], f32, tag="ones_f", name="ones_f")
            nc.vector.memset(ones_f, 1.0)
            ones_b = const.tile([128, 128], bf16, tag="ones_b", name="ones_b")
            nc.vector.memset(ones_b, 1.0)
            eps_t = const.tile([128, 1], f32, tag="eps_t", name="eps_t")
            nc.vector.memset(eps_t, EPS)

            def ln_feature_major(pool, rawpool, ppool, chw, raw, dst_aps):
                """Feature-major LayerNorm core ((x-m)*r over 384 partitions).
                `raw` is a list of CT bf16 [128, chw] APs already produced;
                the normalized result is written directly into `dst_aps`.
                Stats computed in partition-broadcast form via all-ones
                matmuls; stats in f32, broadcast mean and rstd cast to bf16
                so all DVE ops keep uniform dtypes."""
                ps_m = ppool.tile([128, chw], f32, tag="ps_m", name="ps_m")
                for cb in range(CT):
                    nc.tensor.matmul(ps_m, ones_b, raw[cb],
                                     start=(cb == 0), stop=(cb == CT - 1))
                ps_s = ppool.tile([128, chw], f32, tag="ps_s", name="ps_s")
                for cb in range(CT):
                    sq = rawpool.tile([128, chw], bf16, tag="lnsq", name="lnsq")
                    nc.vector.tensor_mul(sq, raw[cb], raw[cb])
                    nc.tensor.matmul(ps_s, ones_b, sq,
                                     start=(cb == 0), stop=(cb == CT - 1))
                m_bf = pool.tile([128, chw], bf16, tag="m_bf", name="m_bf")
                nc.vector.tensor_scalar_mul(m_bf, ps_m, 1.0 / C)
                m2 = pool.tile([128, chw], f32, tag="m2", name="m2")
                nc.vector.tensor_mul(m2, m_bf, m_bf)
                nc.vector.scalar_tensor_tensor(
                    out=m2, in0=ps_s, scalar=1.0 / C, in1=m2,
                    op0=OP.mult, op1=OP.subtract)
                nc.scalar.activation(out=m2, in_=m2, func=AF.Sqrt,
                                     bias=eps_t, scale=1.0)
                r_bc = pool.tile([128, chw], f32, tag="r_bc", name="r_bc")
                nc.vector.reciprocal_approx_fast(out=r_bc, in_=m2)
                r_bf = pool.tile([128, chw], bf16, tag="r_bf", name="r_bf")
                nc.vector.tensor_copy(r_bf, r_bc)
                for cb in range(CT):
                    nc.vector.tensor_sub(raw[cb], raw[cb], m_bf)
                    nc.vector.tensor_mul(dst_aps[cb], raw[cb], r_bf)

            with tc.tile_pool(name="apool", bufs=1) as apool:
                qh = [apool.tile([128, QPC], bf16, tag=f"qh{c}", name=f"qh{c}")
                      for c in range(CT)]
                ql = [apool.tile([128, QPC], bf16, tag=f"ql{c}", name=f"ql{c}")
                      for c in range(CT)]
                oh = [apool.tile([128, QPC], bf16, tag=f"oh{c}", name=f"oh{c}")
                      for c in range(CT)]
                ol = [apool.tile([128, QPC], bf16, tag=f"ol{c}", name=f"ol{c}")
                      for c in range(CT)]

                with tc.tile_pool(name="kvpool", bufs=1) as kvpool:
                    kh = [kvpool.tile([128, L], bf16, tag=f"kh{c}", name=f"kh{c}")
                          for c in range(CT)]
                    kl = [kvpool.tile([128, L], bf16, tag=f"kl{c}", name=f"kl{c}")
                          for c in range(CT)]
                    vh = [kvpool.tile([jn, C], bf16, tag=f"vh{i}", name=f"vh{i}")
                          for i, (_, jn) in enumerate(JTS)]
                    vl = [kvpool.tile([jn, C], bf16, tag=f"vl{i}", name=f"vl{i}")
                          for i, (_, jn) in enumerate(JTS)]
                    kbcol = {}
                    for a in range(2):
                        for h in range(HEADS):
                            kbcol[(a, h)] = kvpool.tile(
                                [128, len(JTS)], f32,
                                tag=f"kbcol{a}{h}", name=f"kbcol{a}{h}")

                    with tc.tile_pool(name="bigpool", bufs=1) as bigpool:
                        p1n = [bigpool.tile([128, L], bf16, tag=f"p1n{c}", name=f"p1n{c}")
                               for c in range(CT)]
                        p2up = [bigpool.tile([128, L], bf16, tag=f"p2up{c}", name=f"p2up{c}")
                                for c in range(CT)]
                        pp = [bigpool.tile([128, L], bf16, tag=f"pp{c}", name=f"pp{c}")
                              for c in range(CT)]
                        xnorm = [bigpool.tile([TT2, C], bf16, tag=f"xnorm{t}", name=f"xnorm{t}")
                                 for t in range(TOK2)]
                        penw3 = load_b3(bigpool, v_penw, "penw")
                        penb3 = load_b3(bigpool, v_penb, "penb")

                        # Phase 1: x = LN_pen_core(p2 @ projT + b), token-major
                        with tc.tile_pool(name="ph1s", bufs=1) as ph1s, \
                             tc.tile_pool(name="ph1t", bufs=3) as ph1, \
                             tc.tile_pool(name="ph1p", bufs=2, space="PSUM") as ph1p:
                            tproj = load_w3(ph1s, w_projT, "projT", rows=2 * C)
                            projb_row = ph1s.tile([1, C], f32r, tag="projb_row", name="projb_row")
                            nc.sync.dma_start(out=projb_row, in_=v_projb)
                            p2s = load_w3(ph1s, p2T, "p2s", rows=2 * C)
                            for tt in range(TOK2):
                                ps = ph1p.tile([TT2, C], f32, tag="ps_x", name="ps_x")
                                sl = slice(tt * TT2, (tt + 1) * TT2)
                                for k in range(6):
                                    nc.tensor.matmul(ps, p2s[k][:, sl], tproj[k],
                                                     start=(k == 0), stop=False)
                                nc.tensor.matmul(ps, r32(ones_f[0:1, 0:TT2]),
                                                 projb_row, start=False, stop=True)
                                st = ph1.tile([TT2, 6], f32, tag="bnst", name="bnst")
                                nc.vector.bn_stats(out=st, in_=ps)
                                mv = ph1.tile([TT2, 2], f32, tag="bnmv", name="bnmv")
                                nc.vector.bn_aggr(out=mv, in_=st)
                                sd = ph1.tile([TT2, 1], f32, tag="sd", name="sd")
                                nc.scalar.activation(out=sd, in_=mv[:, 1:2],
                                                     func=AF.Sqrt,
                                                     bias=eps_t[0:TT2], scale=1.0)
                                rr = ph1.tile([TT2, 1], f32, tag="rr", name="rr")
                                rscr = ph1.tile([TT2, 1], f32, tag="rscr", name="rscr")
                                nc.vector.reciprocal_approx_accurate(
                                    out=rr, in_=sd, scratch=rscr)
                                nmr = ph1.tile([TT2, 1], f32, tag="nmr", name="nmr")
                                nc.vector.scalar_tensor_tensor(
                                    out=nmr, in0=mv[:, 0:1], scalar=-1.0, in1=rr,
                                    op0=OP.mult, op1=OP.mult)
                                nc.scalar.activation(out=xnorm[tt], in_=ps,
                                                     func=AF.Identity,
                                                     bias=nmr, scale=rr)

                        # Merged loop: p2up, p1n, pp chunk by chunk, all SBUF
                        with tc.tile_pool(name="mw", bufs=1) as mw, \
                             tc.tile_pool(name="mt", bufs=1) as mt, \
                             tc.tile_pool(name="mraw", bufs=2) as mraw, \
                             tc.tile_pool(name="mwup", bufs=1) as mwup, \
                             tc.tile_pool(name="mp_up", bufs=2, space="PSUM") as mp_up, \
                             tc.tile_pool(name="mp_st", bufs=1, space="PSUM") as mp_st, \
                             tc.tile_pool(name="mp_pl", bufs=1, space="PSUM") as mp_pl:
                            tl1L = load_w3(mw, w_pl1LT, "pl1LT")
                            tl1R = load_w3(mw, w_pl1RT, "pl1RT")
                            tl2 = load_w3(mw, w_pl2T, "pl2T")
                            bl1 = load_b3(mw, v_pl1b, "pl1b")
                            bl2 = load_b3(mw, v_pl2b, "pl2b")
                            tkh = load_w3(mw, w_khT, "khT")
                            tkl = load_w3(mw, w_klT, "klT")
                            tvh = load_w3(mw, w_vhT, "vhT")
                            tvl = load_w3(mw, w_vlT, "vlT")
                            bqcol = {}
                            for a, dram in ((0, bq_h_col), (1, bq_l_col)):
                                for h in range(HEADS):
                                    t = mw.tile([128, 1], bf16,
                                                tag=f"bqc{a}{h}", name=f"bqc{a}{h}")
                                    nc.sync.dma_start(
                                        out=t, in_=dram[h * 128:(h + 1) * 128, :])
                                    bqcol[(a, h)] = t
                            tqh = load_w3(mw, w_qhT, "qhT")
                            bqh3 = load_b3(mw, v_bqh, "bqh")
                            tql = load_w3(mw, w_qlT, "qlT")
                            bql3 = load_b3(mw, v_bql, "bql")
                            p2upo = [bigpool.tile([128, QC], bf16, tag=f"p2upo{c}", name=f"p2upo{c}")
                                     for c in range(CT)]
                            p1no = [bigpool.tile([128, QC], bf16, tag=f"p1no{c}", name=f"p1no{c}")
                                    for c in range(CT)]
                            v_next = 0

                            def up_raws(wsrc, csl, pstag):
                                # upsample matmuls + pen affine for one chunk
                                wt = mwup.tile([TT2, TOK2 * MC], bf16,
                                               tag="wup_all", name="wup_all")
                                nc.sync.dma_start(
                                    out=wt.rearrange("p (k n) -> p k n", k=TOK2),
                                    in_=wsrc.rearrange("(k p) n -> p k n",
                                                       p=TT2)[:, :, csl])
                                raws = []
                                for cb in range(CT):
                                    ps = mp_up.tile([128, MC], f32, tag=pstag, name=pstag)
                                    for kt in range(TOK2):
                                        nc.tensor.matmul(
                                            ps,
                                            xnorm[kt][:, cb * 128:(cb + 1) * 128],
                                            wt[:, kt * MC:(kt + 1) * MC],
                                            start=(kt == 0), stop=(kt == TOK2 - 1))
                                    r = mraw.tile([128, MC], bf16,
                                                  tag=f"lnraw{cb}", name=f"lnraw{cb}")
                                    nc.vector.tensor_scalar(
                                        out=r, in0=ps, scalar1=penw3[cb],
                                        scalar2=penb3[cb], op0=OP.mult, op1=OP.add)
                                    raws.append(r)
                                return raws

                            def dram_raws(dsrc, csl):
                                # one batched DMA for CT partition-tiles
                                t = mraw.tile([128, CT * MC], bf16,
                                              tag="p1raw", name="p1raw")
                                nc.sync.dma_start(
                                    out=t.rearrange("p (k n) -> p k n", k=CT),
                                    in_=dsrc.rearrange("(k p) n -> p k n",
                                                       p=128)[:, :, csl])
                                return [t[:, i * MC:(i + 1) * MC]
                                        for i in range(CT)]

                            def emit_ph5_chunk(chq):
                                csl = slice(chq * QC, (chq + 1) * QC)
                                raws = up_raws(WupT_own, csl, "ps_up")
                                ln_feature_major(mt, mraw, mp_st, QC, raws,
                                                 p2upo)
                                raws = dram_raws(p1T_own, csl)
                                ln_feature_major(mt, mraw, mp_st, QC, raws,
                                                 p1no)
                                # Q projections for this query chunk
                                for (dst, src, tw, tb) in (
                                        (qh, p1no, tqh, bqh3),
                                        (ql, p2upo, tql, bql3)):
                                    for cb in range(CT):
                                        ps = mp_pl.tile([128, MC], f32,
                                                        tag="ps_k", name="ps_k")
                                        for kt in range(CT):
                                            nc.tensor.matmul(
                                                ps, tw[kt][:, cb * 128:(cb + 1) * 128],
                                                src[kt],
                                                start=(kt == 0),
                                                stop=(kt == CT - 1))
                                        nc.scalar.activation(
                                            out=dst[cb][:, csl], in_=ps,
                                            func=AF.Identity, bias=tb[cb],
                                            scale=1.0)

                            for ch in range(NMC):
                                csl = slice(ch * MC, (ch + 1) * MC)
                                raws = up_raws(WupT, csl, "ps_up")
                                ln_feature_major(mt, mraw, mp_st, MC, raws,
                                                 [p2up[cb][:, csl]
                                                  for cb in range(CT)])
                                raws = dram_raws(p1T, csl)
                                ln_feature_major(mt, mraw, mp_st, MC, raws,
                                                 [p1n[cb][:, csl]
                                                  for cb in range(CT)])
                                gel = []
                                for cb in range(CT):
                                    ps = mp_pl.tile([128, MC], f32, tag="ps_pp", name="ps_pp")
                                    for kt in range(CT):
                                        nc.tensor.matmul(
                                            ps, tl1L[kt][:, cb * 128:(cb + 1) * 128],
                                            p1n[kt][:, csl],
                                            start=(kt == 0), stop=False)
                                    for kt in range(CT):
                                        nc.tensor.matmul(
                                            ps, tl1R[kt][:, cb * 128:(cb + 1) * 128],
                                            p2up[kt][:, csl], start=False,
                                            stop=(kt == CT - 1))
                                    gt = mt.tile([128, MC], bf16, tag=f"gel{cb}", name=f"gel{cb}")
                                    nc.scalar.activation(out=gt, in_=ps,
                                                         func=AF.Gelu,
                                                         bias=bl1[cb], scale=1.0)
                                    gel.append(gt)
                                for cb in range(CT):
                                    ps = mp_pl.tile([128, MC], f32, tag="ps_pp", name="ps_pp")
                                    for kt in range(CT):
                                        nc.tensor.matmul(
                                            ps, tl2[kt][:, cb * 128:(cb + 1) * 128],
                                            gel[kt], start=(kt == 0),
                                            stop=(kt == CT - 1))
                                    nc.vector.tensor_scalar_add(
                                        pp[cb][:, csl], ps, bl2[cb])
                                # inline K projections for this chunk
                                # (kh from p2up, kl from pp)
                                for a, (kk, src, twk) in enumerate(
                                        ((kh, p2up, tkh), (kl, pp, tkl))):
                                    for cb in range(CT):
                                        ps = mp_pl.tile([128, MC], f32,
                                                        tag="ps_k", name="ps_k")
                                        for kt in range(CT):
                                            nc.tensor.matmul(
                                                ps, twk[kt][:, cb * 128:(cb + 1) * 128],
                                                src[kt][:, csl],
                                                start=(kt == 0), stop=(kt == CT - 1))
                                        nc.scalar.activation(
                                            out=kk[cb][:, csl], in_=ps, func=AF.Copy)
                                # V projections + kbcol columns for all
                                # j-tiles fully covered by tokens
                                # [0, (ch+1)*MC): independent PE filler
                                # between the LN dependency chains
                                while v_next < len(JTS) and (
                                        JTS[v_next][0] + JTS[v_next][1]
                                        <= (ch + 1) * MC):
                                    j0, jn = JTS[v_next]
                                    for a, (src, twv, vv, kk) in enumerate(
                                            ((p2up, tvh, vh, kh),
                                             (pp, tvl, vl, kl))):
                                        ps = mp_pl.tile([128, MC], f32,
                                                        tag="ps_v", name="ps_v")
                                        for kt in range(CT):
                                            nc.tensor.matmul(
                                                ps[:jn, 0:C],
                                                src[kt][:, j0:j0 + jn],
                                                twv[kt], start=(kt == 0),
                                                stop=(kt == CT - 1))
                                        nc.scalar.activation(
                                            out=vv[v_next], in_=ps[:jn, 0:C],
                                            func=AF.Copy)
                                        for h in range(HEADS):
                                            ps_c = mp_pl.tile(
                                                [128, 1], f32,
                                                tag="ps_kc", name="ps_kc")
                                            nc.tensor.matmul(
                                                ps_c[:jn], kk[h][:, j0:j0 + jn],
                                                bqcol[(a, h)],
                                                start=True, stop=True)
                                            nc.vector.tensor_copy(
                                                kbcol[(a, h)][:jn,
                                                              v_next:v_next + 1],
                                                ps_c[:jn])
                                    v_next += 1
                                # own-slice recompute interleaved as filler;
                                # chunk 1 feeds only attention qc=1, so it
                                # goes last to pad the merged-loop tail
                                if ch == 4:
                                    emit_ph5_chunk(0)
                                elif ch == NMC - 1:
                                    emit_ph5_chunk(1)



                    # Attention (bigpool freed)
                    with tc.tile_pool(name="atw", bufs=1) as atw:
                        tfoh = load_w3(atw, w_fohT, "fohT", dtype=f32r)
                        bfoh = load_b3(atw, v_fohb, "fohb")
                        tfol = load_w3(atw, w_folT, "folT", dtype=f32r)
                        bfol = load_b3(atw, v_folb, "folb")

                        with tc.tile_pool(name="at", bufs=4) as at, \
                             tc.tile_pool(name="atb", bufs=3) as atb, \
                             tc.tile_pool(name="ato", bufs=1) as ato, \
                             tc.tile_pool(name="atps", bufs=2, space="PSUM") as atps, \
                             tc.tile_pool(name="atpo", bufs=2, space="PSUM") as atpo, \
                             tc.tile_pool(name="atpd", bufs=2, space="PSUM") as atpd, \
                             tc.tile_pool(name="atpp", bufs=2, space="PSUM") as atpp:
                            for qc in range(NQC):
                                qsl = slice(qc * QC, (qc + 1) * QC)
                                onorm = {}
                                for h in range(HEADS):
                                    ps_o = [atpo.tile([128, QC], f32, tag="ps_o", name="ps_o")
                                            for _ in range(2)]
                                    ps_d = [atpd.tile([128, QC], f32, tag="ps_d", name="ps_d")
                                            for _ in range(2)]
                                    for g0, g1 in EBG:
                                        nt = g1 - g0
                                        ebg = atb.tile([128, 5 * QC], bf16,
                                                       tag="ebg", name="ebg")
                                        if JTS[g1 - 1][1] == 128:
                                            nc.sync.dma_start(
                                                out=ebg.rearrange(
                                                    "p (t n) -> p t n",
                                                    t=5)[:, :nt, :],
                                                in_=expB[
                                                    h,
                                                    JTS[g0][0]:JTS[g1 - 1][0] + 128,
                                                    qsl].rearrange(
                                                    "(t p) n -> p t n", p=128))
                                        else:
                                            nc.sync.dma_start(
                                                out=ebg[:JTS[g0][1], 0:QC],
                                                in_=expB[h, JTS[g0][0]:L, qsl])
                                        for i in range(g0, g1):
                                            j0, jn = JTS[i]
                                            eb = ebg[:jn, (i - g0) * QC:
                                                     (i - g0 + 1) * QC]
                                            for a, (kk, qq, vv) in enumerate(
                                                    ((kh, qh, vh), (kl, ql, vl))):
                                                ps_s = atps.tile([jn, QC], f32,
                                                                 tag="ps_s", name="ps_s")
                                                nc.tensor.matmul(
                                                    ps_s, kk[h][:, j0:j0 + jn],
                                                    qq[h][:, qsl],
                                                    start=True, stop=True)
                                                aa = at.tile([jn, QC], bf16, tag="aa", name="aa")
                                                nc.scalar.activation(
                                                    out=aa, in_=ps_s, func=AF.Exp,
                                                    bias=kbcol[(a, h)][:jn, i:i + 1],
                                                    scale=1.0)
                                                nc.vector.tensor_mul(aa, aa, eb)
                                                nc.tensor.matmul(
                                                    ps_o[a],
                                                    vv[i][:, h * 128:(h + 1) * 128],
                                                    aa, start=(i == 0),
                                                    stop=(i == len(JTS) - 1))
                                                nc.tensor.matmul(
                                                    ps_d[a], ones_b[:jn], aa,
                                                    start=(i == 0),
                                                    stop=(i == len(JTS) - 1))
                                    for a in range(2):
                                        rden = at.tile([128, QC], f32, tag="rden", name="rden")
                                        nc.vector.reciprocal_approx_fast(
                                            out=rden, in_=ps_d[a])
                                        on = ato.tile([128, QC], f32r,
                                                      tag=f"on{a}{h}", name=f"on{a}{h}")
                                        nc.vector.tensor_mul(on, ps_o[a], rden)
                                        onorm[(a, h)] = on
                                for a, (dst, tw, tb) in enumerate(
                                        ((oh, tfoh, bfoh), (ol, tfol, bfol))):
                                    for cb in range(CT):
                                        ps = atpp.tile([128, QC], f32,
                                                       tag="ps_fo", name="ps_fo")
                                        for h in range(HEADS):
                                            nc.tensor.matmul(
                                                ps, tw[h][:, cb * 128:(cb + 1) * 128],
                                                onorm[(a, h)],
                                                start=(h == 0),
                                                stop=(h == HEADS - 1))
                                        nc.vector.tensor_scalar_add(
                                            dst[cb][:, qsl], ps, tb[cb])

                # gate, mix, ff
                with tc.tile_pool(name="ph8w", bufs=1) as ph8w, \
                     tc.tile_pool(name="ph8", bufs=2) as ph8, \
                     tc.tile_pool(name="ph8p", bufs=2, space="PSUM") as ph8p:
                    tg1L = load_w3(ph8w, w_g1LT, "g1LT")
                    tg1R = load_w3(ph8w, w_g1RT, "g1RT")
                    bg1 = load_b3(ph8w, v_g1b, "g1b")
                    tg2 = load_w3(ph8w, w_g2T, "g2T")
                    g2b_t = ph8w.tile([1, 1], f32, tag="g2b_t", name="g2b_t")
                    nc.sync.dma_start(
                        out=g2b_t, in_=v_g2b.rearrange("(a b) -> a b", a=1))
                    tffL = load_w3(ph8w, w_ffLT, "ffLT")
                    tffP = load_w3(ph8w, w_ffPT, "ffPT")
                    bff = load_b3(ph8w, v_ffb, "ffb")
                    p1o = [ph8w.tile([128, QPC], bf16, tag=f"p1o{c}", name=f"p1o{c}")
                           for c in range(CT)]
                    for cb in range(CT):
                        nc.sync.dma_start(
                            out=p1o[cb],
                            in_=p1T_own[cb * 128:(cb + 1) * 128, :])
                    for qc in range(NQC):
                        qsl = slice(qc * QC, (qc + 1) * QC)
                        gel = []
                        for cb in range(CT):
                            ps = ph8p.tile([128, QC], f32, tag="ps_g1", name="ps_g1")
                            for kt in range(CT):
                                nc.tensor.matmul(
                                    ps, tg1L[kt][:, cb * 128:(cb + 1) * 128],
                                    oh[kt][:, qsl],
                                    start=(kt == 0), stop=False)
                            for kt in range(CT):
                                nc.tensor.matmul(
                                    ps, tg1R[kt][:, cb * 128:(cb + 1) * 128],
                                    ol[kt][:, qsl], start=False,
                                    stop=(kt == CT - 1))
                            gt = ph8.tile([128, QC], bf16, tag=f"ggel{cb}", name=f"ggel{cb}")
                            nc.scalar.activation(out=gt, in_=ps, func=AF.Gelu,
                                                 bias=bg1[cb], scale=1.0)
                            gel.append(gt)
                        ps_z = ph8p.tile([1, QC], f32, tag="ps_z", name="ps_z")
                        for kt in range(CT):
                            nc.tensor.matmul(ps_z, tg2[kt], gel[kt],
                                             start=(kt == 0),
                                             stop=(kt == CT - 1))
                        gate = ph8.tile([1, QC], f32r, tag="gate", name="gate")
                        nc.scalar.activation(out=gate, in_=ps_z,
                                             func=AF.Sigmoid,
                                             bias=g2b_t, scale=1.0)
                        ps_gb = ph8p.tile([128, QC], f32, tag="ps_gb", name="ps_gb")
                        nc.tensor.matmul(ps_gb, r32(ones_f[0:1, :]), gate,
                                         start=True, stop=True)
                        gb_bf = ph8.tile([128, QC], bf16, tag="gb_bf", name="gb_bf")
                        nc.vector.tensor_copy(gb_bf, ps_gb)
                        mix = []
                        for cb in range(CT):
                            dd = ph8.tile([128, QC], bf16, tag="dd", name="dd")
                            nc.vector.tensor_sub(dd, oh[cb][:, qsl],
                                                 ol[cb][:, qsl])
                            d2 = ph8.tile([128, QC], bf16, tag="d2", name="d2")
                            nc.vector.tensor_mul(d2, dd, gb_bf)
                            mx = ph8.tile([128, QC], bf16, tag=f"mix{cb}", name=f"mix{cb}")
                            nc.vector.tensor_add(mx, d2, ol[cb][:, qsl])
                            mix.append(mx)
                        for cb in range(CT):
                            ps = ph8p.tile([128, QC], f32, tag="ps_ff", name="ps_ff")
                            for kt in range(CT):
                                nc.tensor.matmul(
                                    ps, tffL[kt][:, cb * 128:(cb + 1) * 128],
                                    mix[kt], start=(kt == 0), stop=False)
                            for kt in range(CT):
                                nc.tensor.matmul(
                                    ps, tffP[kt][:, cb * 128:(cb + 1) * 128],
                                    p1o[kt][:, qsl], start=False,
                                    stop=(kt == CT - 1))
                            res = ph8.tile([128, QC], f32, tag="res", name="res")
                            nc.vector.tensor_scalar_add(res, ps, bff[cb])
                            nc.sync.dma_start(
                                out=outT[cb * 128:(cb + 1) * 128, qsl],
                                in_=res)

    nc.compile()
    return nc


def _prepare(inputs):
    """Host prep + input sharding. Returns (nc, in_maps)."""
    global _COMPILED
    import ml_dtypes
    bf = ml_dtypes.bfloat16
    inp = {k: np.asarray(v) for k, v in inputs.items()}
    g = _host_prep(inp)

    if _COMPILED is None:
        _COMPILED = _build()
    nc = _COMPILED

    p1 = inp["p1"].astype(np.float32)
    p2 = inp["p2"].astype(np.float32)

    # per-head scaled q-biases for the kb bias row (column layout)
    bq_h_col = g["bqh"].reshape(C, 1).astype(bf)
    bq_l_col = g["bql"].reshape(C, 1).astype(bf)

    shared = {
        "WupT": g["WupT"].astype(bf),
        "w_projT": g["projT"].astype(bf),
        "v_projb": g["projb"].reshape(1, C),
        "v_penw": g["penw"], "v_penb": g["penb"],
        "w_qhT": g["wqhT"].astype(bf), "v_bqh": g["bqh"],
        "w_qlT": g["wqlT"].astype(bf), "v_bql": g["bql"],
        "w_khT": g["wkhT"].astype(bf), "w_klT": g["wklT"].astype(bf),
        "w_vhT": g["wvhT"].astype(bf), "w_vlT": g["wvlT"].astype(bf),
        "bq_h_col": bq_h_col, "bq_l_col": bq_l_col,
        "w_pl1LT": g["pl1LT"].astype(bf), "w_pl1RT": g["pl1RT"].astype(bf),
        "v_pl1b": g["pl1b"],
        "w_pl2T": g["pl2T"].astype(bf), "v_pl2b": g["pl2b"],
        "w_fohT": g["fohT"], "v_fohb": g["fohb"],
        "w_folT": g["folT"], "v_folb": g["folb"],
        "w_g1LT": g["g1LT"].astype(bf), "w_g1RT": g["g1RT"].astype(bf),
        "v_g1b": g["g1b"],
        "w_g2T": g["g2T"].astype(bf), "v_g2b": g["g2b"],
        "w_ffLT": g["ffLT"].astype(bf), "w_ffPT": g["ffPT"].astype(bf),
        "v_ffb": g["ffb"],
    }
    shared = {k: np.ascontiguousarray(v) for k, v in shared.items()}

    in_maps = []
    for core in range(NCORES):
        b, qi = divmod(core, 4)
        q0 = qi * QPC
        m = dict(shared)
        m["p1T"] = np.ascontiguousarray(p1[b].T.astype(bf))
        m["p1T_own"] = np.ascontiguousarray(p1[b, q0:q0 + QPC, :].T.astype(bf))
        m["p2T"] = np.ascontiguousarray(p2[b].T.astype(bf))
        m["WupT_own"] = np.ascontiguousarray(g["WupT"][:, q0:q0 + QPC].astype(bf))
        m["expB"] = np.ascontiguousarray(g["expB"][:, :, q0:q0 + QPC])
        in_maps.append(m)

    return nc, in_maps


def _run(nc, in_maps):
    from concourse.bass_utils import run_bass_kernel_spmd
    res = run_bass_kernel_spmd(nc, in_maps, core_ids=list(range(NCORES)))
    out = np.zeros((B, L, C), np.float32)
    for core in range(NCORES):
        b, qi = divmod(core, 4)
        q0 = qi * QPC
        out[b, q0:q0 + QPC, :] = res.results[core]["outT"].T
    return out


def kernel(**inputs):
    nc, in_maps = _prepare(inputs)
    return _run(nc, in_maps)


# revision 67
# speedup vs baseline: 1.0540x; 1.0281x over previous
"""Trainium2 Bass kernel for nn_CrossfusionBidirectional.

Sharding: 8 cores = (batch b in {0,1}) x (query-row quarter qi in {0..3}).
Each core computes output rows [qi*784, (qi+1)*784) of batch b with zero
cross-core communication; the host concatenates the 8 slices.

v2: bf16 dataflow. All weights and SBUF activations are bf16 (PSUM stays
f32, LayerNorm statistics in f32); the pre-attention stages run as one
merged per-chunk pipeline with p2up/pp kept in SBUF (no DRAM roundtrip).
Attention is computed transposed (S^T[j, q]) with multiplicative rel-pos
bias exp(s + kb) * exp(B); softmax denominators come from an all-ones
matmul whose output is already broadcast across partitions. LayerNorm
affine params and gammas are folded into downstream weights on the host;
K-projection biases drop out via softmax shift invariance; Q-projection
biases enter through the per-key exp bias column; V-projection biases fold
into the output-projection bias because softmax rows sum to one.
"""

import numpy as np

B, L, C, HEADS = 2, 3136, 384, 3
H, H2 = 56, 28
L2 = L // 4
HD = C // HEADS
EPS = 1e-5
NCORES = 8
QPC = L // 4          # 784 query rows per core
CT = C // 128         # 3 feature tiles
NMC, MC = 8, 392      # merged-loop chunking of full L
NQC, QC = 2, 392      # per-core query chunking
TOK2, TT2 = 7, 112    # low-res token tiling (784 = 7*112)
JTS = [(i * 128, 128) for i in range(24)] + [(3072, 64)]   # key tiles
EBG = [(0, 5), (5, 10), (10, 15), (15, 20), (20, 24), (24, 25)]  # eb DMA groups

_COMPILED = None


def _resize_weight_mat(n_in, n_out):
    # jax.image.resize 'linear' half-pixel: triangle kernel, normalized
    scale = n_out / n_in
    sample_f = (np.arange(n_out) + 0.5) / scale - 0.5
    w = 1.0 - np.abs(sample_f[:, None] - np.arange(n_in)[None, :])
    w = np.clip(w, 0.0, 1.0)
    w = w / w.sum(axis=1, keepdims=True)
    return w.astype(np.float32)


def _host_prep(inp):
    f32 = np.float32
    g = {}
    scale = f32(HD ** -0.5)
    n1w, n1b = inp["n1_w"].astype(f32), inp["n1_b"].astype(f32)
    n2w, n2b = inp["n2_w"].astype(f32), inp["n2_b"].astype(f32)

    def fold_in(w, b, lnw, lnb):
        return (w * lnw[None, :]).astype(f32), (b + w @ lnb).astype(f32)

    wqh, bqh = fold_in(inp["wqh_w"], inp["wqh_b"], n2w, n2b)
    wkh, _ = fold_in(inp["wkh_w"], inp["wkh_b"], n1w, n1b)
    wvh, bvh = fold_in(inp["wvh_w"], inp["wvh_b"], n1w, n1b)
    wql, bql = fold_in(inp["wql_w"], inp["wql_b"], n1w, n1b)
    wkl = inp["wkl_w"].astype(f32)
    wvl, bvl = inp["wvl_w"].astype(f32), inp["wvl_b"].astype(f32)

    g["wqhT"], g["bqh"] = (wqh.T * scale).copy(), bqh * scale
    g["wqlT"], g["bql"] = (wql.T * scale).copy(), bql * scale
    g["wkhT"], g["wklT"] = wkh.T.copy(), wkl.T.copy()
    g["wvhT"], g["wvlT"] = wvh.T.copy(), wvl.T.copy()

    pl1L, pl1R = inp["pl1_w"][:, :C], inp["pl1_w"][:, C:]
    pl1Lw, _ = fold_in(pl1L, np.zeros(C, f32), n2w, n2b)
    pl1Rw, _ = fold_in(pl1R, np.zeros(C, f32), n1w, n1b)
    g["pl1LT"], g["pl1RT"] = pl1Lw.T.copy(), pl1Rw.T.copy()
    g["pl1b"] = (inp["pl1_b"] + pl1L @ n2b + pl1R @ n1b).astype(f32)
    g["pl2T"], g["pl2b"] = inp["pl2_w"].T.copy(), inp["pl2_b"].astype(f32)

    gh, gl = f32(inp["gamma_h"][0]), f32(inp["gamma_l"][0])
    g["fohT"] = (inp["foh_w"].T * gh).astype(f32)
    g["fohb"] = ((inp["foh_b"] + inp["foh_w"] @ bvh) * gh).astype(f32)
    g["folT"] = (inp["fol_w"].T * gl).astype(f32)
    g["folb"] = ((inp["fol_b"] + inp["fol_w"] @ bvl) * gl).astype(f32)

    g["g1LT"] = inp["g1_w"][:, :C].T.copy().astype(f32)
    g["g1RT"] = inp["g1_w"][:, C:].T.copy().astype(f32)
    g["g1b"] = inp["g1_b"].astype(f32)
    g["g2T"] = inp["g2_w"].T.copy().astype(f32)   # [384, 1]
    g["g2b"] = inp["g2_b"].astype(f32)            # [1]

    ffL, ffR = inp["ff_w"][:, :C], inp["ff_w"][:, C:]
    g["ffLT"] = ffL.T.copy().astype(f32)
    g["ffPT"] = (ffL + ffR).T.copy().astype(f32)
    g["ffb"] = inp["ff_b"].astype(f32)

    g["projT"] = inp["proj_w"].T.copy().astype(f32)
    g["projb"] = inp["proj_b"].astype(f32)
    g["penw"], g["penb"] = inp["pen_w"].astype(f32), inp["pen_b"].astype(f32)

    wr = _resize_weight_mat(H2, H)
    g["WupT"] = np.kron(wr, wr).T.copy().astype(f32)  # [784, 3136]

    import ml_dtypes
    expt = np.exp(inp["rpb_table"].astype(f32))       # [12321, 3]
    rel = np.asarray(inp["rel_index"])                # [L, L] int32 (rel[i, j])
    g["expB"] = np.ascontiguousarray(
        expt[rel.T].transpose(2, 0, 1)).astype(ml_dtypes.bfloat16)
    return g


def _build():
    import contextlib
    import concourse.bass as bass  # noqa: F401
    import concourse.tile as tile
    from concourse import bacc, mybir

    f32, bf16, f32r = mybir.dt.float32, mybir.dt.bfloat16, mybir.dt.float32r
    AF = mybir.ActivationFunctionType
    OP = mybir.AluOpType

    nc = bacc.Bacc("TRN2", target_bir_lowering=False, debug=False,
                   num_devices=NCORES)

    def din(name, shape, dtype=f32):
        return nc.dram_tensor(name, shape, dtype, kind="ExternalInput").ap()

    p1T = din("p1T", [C, L], bf16)
    p1T_own = din("p1T_own", [C, QPC], bf16)
    p2T = din("p2T", [2 * C, L2], bf16)
    WupT = din("WupT", [L2, L], bf16)
    WupT_own = din("WupT_own", [L2, QPC], bf16)
    expB = din("expB", [HEADS, L, QPC], bf16)
    w_projT = din("w_projT", [2 * C, C], bf16)
    v_projb = din("v_projb", [1, C], f32r)
    v_penw, v_penb = din("v_penw", [C]), din("v_penb", [C])
    w_qhT, v_bqh = din("w_qhT", [C, C], bf16), din("v_bqh", [C])
    w_qlT, v_bql = din("w_qlT", [C, C], bf16), din("v_bql", [C])
    w_khT, w_klT = din("w_khT", [C, C], bf16), din("w_klT", [C, C], bf16)
    w_vhT, w_vlT = din("w_vhT", [C, C], bf16), din("w_vlT", [C, C], bf16)
    bq_h_col = din("bq_h_col", [C, 1], bf16)   # scaled q-bias, head-major
    bq_l_col = din("bq_l_col", [C, 1], bf16)
    w_pl1LT, w_pl1RT = din("w_pl1LT", [C, C], bf16), din("w_pl1RT", [C, C], bf16)
    v_pl1b = din("v_pl1b", [C])
    w_pl2T, v_pl2b = din("w_pl2T", [C, C], bf16), din("v_pl2b", [C])
    w_fohT, v_fohb = din("w_fohT", [C, C], f32r), din("v_fohb", [C])
    w_folT, v_folb = din("w_folT", [C, C], f32r), din("v_folb", [C])
    w_g1LT, w_g1RT = din("w_g1LT", [C, C], bf16), din("w_g1RT", [C, C], bf16)
    v_g1b = din("v_g1b", [C])
    w_g2T, v_g2b = din("w_g2T", [C, 1], bf16), din("v_g2b", [1])
    w_ffLT, w_ffPT = din("w_ffLT", [C, C], bf16), din("w_ffPT", [C, C], bf16)
    v_ffb = din("v_ffb", [C])

    outT = nc.dram_tensor("outT", [C, QPC], f32, kind="ExternalOutput").ap()

    def r32(ap):
        return ap.bitcast(f32r)

    with tile.TileContext(nc) as tc:
        with tc.tile_pool(name="const", bufs=1) as const:
            def load_w3(pool, dram, tag, rows=C, dtype=bf16):
                # one batched DMA per weight matrix: partition-tiles land
                # side by side on the free dim
                k = rows // 128
                n = dram.shape[1]
                t = pool.tile([128, k * n], dtype, tag=tag, name=tag)
                nc.sync.dma_start(
                    out=t.rearrange("p (k n) -> p k n", k=k),
                    in_=dram.rearrange("(k p) n -> p k n", p=128))
                return [t[:, i * n:(i + 1) * n] for i in range(k)]

            def load_b3(pool, dram, tag, dtype=f32):
                t = pool.tile([128, CT], dtype, tag=tag, name=tag)
                nc.sync.dma_start(
                    out=t,
                    in_=dram.bitcast(dtype).rearrange("(k p) -> p k", p=128))
                return [t[:, i:i + 1] for i in range(CT)]

            ones_f = const.tile([128, 128], f32, tag="ones_f", name="ones_f")
            nc.vector.memset(ones_f, 1.0)
            ones_b = const.tile([128, 128], bf16, tag="ones_b", name="ones_b")
            nc.vector.memset(ones_b, 1.0)
            eps_t = const.tile([128, 1], f32, tag="eps_t", name="eps_t")
            nc.vector.memset(eps_t, EPS)

            def ln_feature_major(pool, rawpool, ppool, chw, raw, dst_aps):
                """Feature-major LayerNorm core ((x-m)*r over 384 partitions).
                `raw` is a list of CT bf16 [128, chw] APs already produced;
                the normalized result is written directly into `dst_aps`.
                Stats computed in partition-broadcast form via all-ones
                matmuls; stats in f32, broadcast mean and rstd cast to bf16
                so all DVE ops keep uniform dtypes."""
                ps_m = ppool.tile([128, chw], f32, tag="ps_m", name="ps_m")
                for cb in range(CT):
                    nc.tensor.matmul(ps_m, ones_b, raw[cb],
                                     start=(cb == 0), stop=(cb == CT - 1))
                ps_s = ppool.tile([128, chw], f32, tag="ps_s", name="ps_s")
                for cb in range(CT):
                    sq = rawpool.tile([128, chw], bf16, tag="lnsq", name="lnsq")
                    nc.vector.tensor_mul(sq, raw[cb], raw[cb])
                    nc.tensor.matmul(ps_s, ones_b, sq,
                                     start=(cb == 0), stop=(cb == CT - 1))
                m_bf = pool.tile([128, chw], bf16, tag="m_bf", name="m_bf")
                nc.vector.tensor_scalar_mul(m_bf, ps_m, 1.0 / C)
                m2 = pool.tile([128, chw], f32, tag="m2", name="m2")
                nc.vector.tensor_mul(m2, m_bf, m_bf)
                nc.vector.scalar_tensor_tensor(
                    out=m2, in0=ps_s, scalar=1.0 / C, in1=m2,
                    op0=OP.mult, op1=OP.subtract)
                nc.scalar.activation(out=m2, in_=m2, func=AF.Sqrt,
                                     bias=eps_t, scale=1.0)
                r_bc = pool.tile([128, chw], f32, tag="r_bc", name="r_bc")
                nc.vector.reciprocal_approx_fast(out=r_bc, in_=m2)
                r_bf = pool.tile([128, chw], bf16, tag="r_bf", name="r_bf")
                nc.vector.tensor_copy(r_bf, r_bc)
                for cb in range(CT):
                    nc.vector.tensor_sub(raw[cb], raw[cb], m_bf)
                    nc.vector.tensor_mul(dst_aps[cb], raw[cb], r_bf)

            with tc.tile_pool(name="apool", bufs=1) as apool:
                qh = [apool.tile([128, QPC], bf16, tag=f"qh{c}", name=f"qh{c}")
                      for c in range(CT)]
                ql = [apool.tile([128, QPC], bf16, tag=f"ql{c}", name=f"ql{c}")
                      for c in range(CT)]
                oh = [apool.tile([128, QPC], bf16, tag=f"oh{c}", name=f"oh{c}")
                      for c in range(CT)]
                ol = [apool.tile([128, QPC], bf16, tag=f"ol{c}", name=f"ol{c}")
                      for c in range(CT)]

                with tc.tile_pool(name="kvpool", bufs=1) as kvpool:
                    kh = [kvpool.tile([128, L], bf16, tag=f"kh{c}", name=f"kh{c}")
                          for c in range(CT)]
                    kl = [kvpool.tile([128, L], bf16, tag=f"kl{c}", name=f"kl{c}")
                          for c in range(CT)]
                    vh = [kvpool.tile([jn, C], bf16, tag=f"vh{i}", name=f"vh{i}")
                          for i, (_, jn) in enumerate(JTS)]
                    vl = [kvpool.tile([jn, C], bf16, tag=f"vl{i}", name=f"vl{i}")
                          for i, (_, jn) in enumerate(JTS)]
                    kbcol = {}
                    for a in range(2):
                        for h in range(HEADS):
                            kbcol[(a, h)] = kvpool.tile(
                                [128, len(JTS)], f32,
                                tag=f"kbcol{a}{h}", name=f"kbcol{a}{h}")

                    with tc.tile_pool(name="bigpool", bufs=1) as bigpool:
                        p1n = [bigpool.tile([128, L], bf16, tag=f"p1n{c}", name=f"p1n{c}")
                               for c in range(CT)]
                        p2up = [bigpool.tile([128, L], bf16, tag=f"p2up{c}", name=f"p2up{c}")
                                for c in range(CT)]
                        pp = [bigpool.tile([128, L], bf16, tag=f"pp{c}", name=f"pp{c}")
                              for c in range(CT)]
                        xnorm = [bigpool.tile([TT2, C], bf16, tag=f"xnorm{t}", name=f"xnorm{t}")
                                 for t in range(TOK2)]
                        penw3 = load_b3(bigpool, v_penw, "penw")
                        penb3 = load_b3(bigpool, v_penb, "penb")

                        # Phase 1: x = LN_pen_core(p2 @ projT + b), token-major
                        with tc.tile_pool(name="ph1s", bufs=1) as ph1s, \
                             tc.tile_pool(name="ph1t", bufs=3) as ph1, \
                             tc.tile_pool(name="ph1p", bufs=2, space="PSUM") as ph1p:
                            tproj = load_w3(ph1s, w_projT, "projT", rows=2 * C)
                            projb_row = ph1s.tile([1, C], f32r, tag="projb_row", name="projb_row")
                            nc.sync.dma_start(out=projb_row, in_=v_projb)
                            p2s = load_w3(ph1s, p2T, "p2s", rows=2 * C)
                            for tt in range(TOK2):
                                ps = ph1p.tile([TT2, C], f32, tag="ps_x", name="ps_x")
                                sl = slice(tt * TT2, (tt + 1) * TT2)
                                for k in range(6):
                                    nc.tensor.matmul(ps, p2s[k][:, sl], tproj[k],
                                                     start=(k == 0), stop=False)
                                nc.tensor.matmul(ps, r32(ones_f[0:1, 0:TT2]),
                                                 projb_row, start=False, stop=True)
                                st = ph1.tile([TT2, 6], f32, tag="bnst", name="bnst")
                                nc.vector.bn_stats(out=st, in_=ps)
                                mv = ph1.tile([TT2, 2], f32, tag="bnmv", name="bnmv")
                                nc.vector.bn_aggr(out=mv, in_=st)
                                sd = ph1.tile([TT2, 1], f32, tag="sd", name="sd")
                                nc.scalar.activation(out=sd, in_=mv[:, 1:2],
                                                     func=AF.Sqrt,
                                                     bias=eps_t[0:TT2], scale=1.0)
                                rr = ph1.tile([TT2, 1], f32, tag="rr", name="rr")
                                rscr = ph1.tile([TT2, 1], f32, tag="rscr", name="rscr")
                                nc.vector.reciprocal_approx_accurate(
                                    out=rr, in_=sd, scratch=rscr)
                                nmr = ph1.tile([TT2, 1], f32, tag="nmr", name="nmr")
                                nc.vector.scalar_tensor_tensor(
                                    out=nmr, in0=mv[:, 0:1], scalar=-1.0, in1=rr,
                                    op0=OP.mult, op1=OP.mult)
                                nc.scalar.activation(out=xnorm[tt], in_=ps,
                                                     func=AF.Identity,
                                                     bias=nmr, scale=rr)

                        # Merged loop: p2up, p1n, pp chunk by chunk, all SBUF
                        with tc.tile_pool(name="mw", bufs=1) as mw, \
                             tc.tile_pool(name="mt", bufs=1) as mt, \
                             tc.tile_pool(name="mraw", bufs=2) as mraw, \
                             tc.tile_pool(name="mwup", bufs=1) as mwup, \
                             tc.tile_pool(name="mp_up", bufs=2, space="PSUM") as mp_up, \
                             tc.tile_pool(name="mp_st", bufs=1, space="PSUM") as mp_st, \
                             tc.tile_pool(name="mp_pl", bufs=1, space="PSUM") as mp_pl:
                            tl1L = load_w3(mw, w_pl1LT, "pl1LT")
                            tl1R = load_w3(mw, w_pl1RT, "pl1RT")
                            tl2 = load_w3(mw, w_pl2T, "pl2T")
                            bl1 = load_b3(mw, v_pl1b, "pl1b")
                            bl2 = load_b3(mw, v_pl2b, "pl2b")
                            tkh = load_w3(mw, w_khT, "khT")
                            tkl = load_w3(mw, w_klT, "klT")
                            tvh = load_w3(mw, w_vhT, "vhT")
                            tvl = load_w3(mw, w_vlT, "vlT")
                            bqcol = {}
                            for a, dram in ((0, bq_h_col), (1, bq_l_col)):
                                for h in range(HEADS):
                                    t = mw.tile([128, 1], bf16,
                                                tag=f"bqc{a}{h}", name=f"bqc{a}{h}")
                                    nc.sync.dma_start(
                                        out=t, in_=dram[h * 128:(h + 1) * 128, :])
                                    bqcol[(a, h)] = t
                            tqh = load_w3(mw, w_qhT, "qhT")
                            bqh3 = load_b3(mw, v_bqh, "bqh")
                            tql = load_w3(mw, w_qlT, "qlT")
                            bql3 = load_b3(mw, v_bql, "bql")
                            p2upo = [bigpool.tile([128, QC], bf16, tag=f"p2upo{c}", name=f"p2upo{c}")
                                     for c in range(CT)]
                            p1no = [bigpool.tile([128, QC], bf16, tag=f"p1no{c}", name=f"p1no{c}")
                                    for c in range(CT)]
                            v_next = 0

                            def up_raws(wsrc, csl, pstag):
                                # upsample matmuls + pen affine for one chunk
                                wt = mwup.tile([TT2, TOK2 * MC], bf16,
                                               tag="wup_all", name="wup_all")
                                nc.sync.dma_start(
                                    out=wt.rearrange("p (k n) -> p k n", k=TOK2),
                                    in_=wsrc.rearrange("(k p) n -> p k n",
                                                       p=TT2)[:, :, csl])
                                raws = []
                                for cb in range(CT):
                                    ps = mp_up.tile([128, MC], f32, tag=pstag, name=pstag)
                                    for kt in range(TOK2):
                                        nc.tensor.matmul(
                                            ps,
                                            xnorm[kt][:, cb * 128:(cb + 1) * 128],
                                            wt[:, kt * MC:(kt + 1) * MC],
                                            start=(kt == 0), stop=(kt == TOK2 - 1))
                                    r = mraw.tile([128, MC], bf16,
                                                  tag=f"lnraw{cb}", name=f"lnraw{cb}")
                                    nc.vector.tensor_scalar(
                                        out=r, in0=ps, scalar1=penw3[cb],
                                        scalar2=penb3[cb], op0=OP.mult, op1=OP.add)
                                    raws.append(r)
                                return raws

                            def dram_raws(dsrc, csl):
                                # one batched DMA for CT partition-tiles
                                t = mraw.tile([128, CT * MC], bf16,
                                              tag="p1raw", name="p1raw")
                                nc.sync.dma_start(
                                    out=t.rearrange("p (k n) -> p k n", k=CT),
                                    in_=dsrc.rearrange("(k p) n -> p k n",
                                                       p=128)[:, :, csl])
                                return [t[:, i * MC:(i + 1) * MC]
                                        for i in range(CT)]

                            def emit_ph5_chunk(chq):
                                csl = slice(chq * QC, (chq + 1) * QC)
                                raws = up_raws(WupT_own, csl, "ps_up")
                                ln_feature_major(mt, mraw, mp_st, QC, raws,
                                                 p2upo)
                                raws = dram_raws(p1T_own, csl)
                                ln_feature_major(mt, mraw, mp_st, QC, raws,
                                                 p1no)
                                # Q projections for this query chunk
                                for (dst, src, tw, tb) in (
                                        (qh, p1no, tqh, bqh3),
                                        (ql, p2upo, tql, bql3)):
                                    for cb in range(CT):
                                        ps = mp_pl.tile([128, MC], f32,
                                                        tag="ps_k", name="ps_k")
                                        for kt in range(CT):
                                            nc.tensor.matmul(
                                                ps, tw[kt][:, cb * 128:(cb + 1) * 128],
                                                src[kt],
                                                start=(kt == 0),
                                                stop=(kt == CT - 1))
                                        nc.scalar.activation(
                                            out=dst[cb][:, csl], in_=ps,
                                            func=AF.Identity, bias=tb[cb],
                                            scale=1.0)

                            for ch in range(NMC):
                                csl = slice(ch * MC, (ch + 1) * MC)
                                raws = up_raws(WupT, csl, "ps_up")
                                ln_feature_major(mt, mraw, mp_st, MC, raws,
                                                 [p2up[cb][:, csl]
                                                  for cb in range(CT)])
                                raws = dram_raws(p1T, csl)
                                ln_feature_major(mt, mraw, mp_st, MC, raws,
                                                 [p1n[cb][:, csl]
                                                  for cb in range(CT)])
                                gel = []
                                for cb in range(CT):
                                    ps = mp_pl.tile([128, MC], f32, tag="ps_pp", name="ps_pp")
                                    for kt in range(CT):
                                        nc.tensor.matmul(
                                            ps, tl1L[kt][:, cb * 128:(cb + 1) * 128],
                                            p1n[kt][:, csl],
                                            start=(kt == 0), stop=False)
                                    for kt in range(CT):
                                        nc.tensor.matmul(
                                            ps, tl1R[kt][:, cb * 128:(cb + 1) * 128],
                                            p2up[kt][:, csl], start=False,
                                            stop=(kt == CT - 1))
                                    gt = mt.tile([128, MC], bf16, tag=f"gel{cb}", name=f"gel{cb}")
                                    nc.scalar.activation(out=gt, in_=ps,
                                                         func=AF.Gelu,
                                                         bias=bl1[cb], scale=1.0)
                                    gel.append(gt)
                                for cb in range(CT):
                                    ps = mp_pl.tile([128, MC], f32, tag="ps_pp", name="ps_pp")
                                    for kt in range(CT):
                                        nc.tensor.matmul(
                                            ps, tl2[kt][:, cb * 128:(cb + 1) * 128],
                                            gel[kt], start=(kt == 0),
                                            stop=(kt == CT - 1))
                                    nc.vector.tensor_scalar_add(
                                        pp[cb][:, csl], ps, bl2[cb])
                                # inline K projections for this chunk
                                # (kh from p2up, kl from pp)
                                for a, (kk, src, twk) in enumerate(
                                        ((kh, p2up, tkh), (kl, pp, tkl))):
                                    for cb in range(CT):
                                        ps = mp_pl.tile([128, MC], f32,
                                                        tag="ps_k", name="ps_k")
                                        for kt in range(CT):
                                            nc.tensor.matmul(
                                                ps, twk[kt][:, cb * 128:(cb + 1) * 128],
                                                src[kt][:, csl],
                                                start=(kt == 0), stop=(kt == CT - 1))
                                        nc.scalar.activation(
                                            out=kk[cb][:, csl], in_=ps, func=AF.Copy)
                                # V projections + kbcol columns for all
                                # j-tiles fully covered by tokens
                                # [0, (ch+1)*MC): independent PE filler
                                # between the LN dependency chains
                                while v_next < len(JTS) and (
                                        JTS[v_next][0] + JTS[v_next][1]
                                        <= (ch + 1) * MC):
                                    j0, jn = JTS[v_next]
                                    for a, (src, twv, vv, kk) in enumerate(
                                            ((p2up, tvh, vh, kh),
                                             (pp, tvl, vl, kl))):
                                        ps = mp_pl.tile([128, MC], f32,
                                                        tag="ps_v", name="ps_v")
                                        for kt in range(CT):
                                            nc.tensor.matmul(
                                                ps[:jn, 0:C],
                                                src[kt][:, j0:j0 + jn],
                                                twv[kt], start=(kt == 0),
                                                stop=(kt == CT - 1))
                                        nc.scalar.activation(
                                            out=vv[v_next], in_=ps[:jn, 0:C],
                                            func=AF.Copy)
                                        for h in range(HEADS):
                                            ps_c = mp_pl.tile(
                                                [128, 1], f32,
                                                tag="ps_kc", name="ps_kc")
                                            nc.tensor.matmul(
                                                ps_c[:jn], kk[h][:, j0:j0 + jn],
                                                bqcol[(a, h)],
                                                start=True, stop=True)
                                            nc.vector.tensor_copy(
                                                kbcol[(a, h)][:jn,
                                                              v_next:v_next + 1],
                                                ps_c[:jn])
                                    v_next += 1
                                # own-slice recompute interleaved as filler;
                                # chunk 1 feeds only attention qc=1, so it
                                # goes last to pad the merged-loop tail
                                if ch == 4:
                                    emit_ph5_chunk(0)
                                elif ch == NMC - 1:
                                    emit_ph5_chunk(1)



                    # Attention (bigpool freed)
                    with tc.tile_pool(name="atw", bufs=1) as atw:
                        tfoh = load_w3(atw, w_fohT, "fohT", dtype=f32r)
                        bfoh = load_b3(atw, v_fohb, "fohb")
                        tfol = load_w3(atw, w_folT, "folT", dtype=f32r)
                        bfol = load_b3(atw, v_folb, "folb")

                        with tc.tile_pool(name="at", bufs=4) as at, \
                             tc.tile_pool(name="atb", bufs=3) as atb, \
                             tc.tile_pool(name="accp", bufs=2) as accp, \
                             tc.tile_pool(name="ato", bufs=1) as ato, \
                             tc.tile_pool(name="atps", bufs=2, space="PSUM") as atps, \
                             tc.tile_pool(name="atpo", bufs=2, space="PSUM") as atpo, \
                             tc.tile_pool(name="atpd", bufs=2, space="PSUM") as atpd, \
                             tc.tile_pool(name="atpp", bufs=2, space="PSUM") as atpp:
                            for qc in range(NQC):
                                qsl = slice(qc * QC, (qc + 1) * QC)
                                onorm = {}
                                for h in range(HEADS):
                                    ps_o = [atpo.tile([128, QC], f32, tag="ps_o", name="ps_o")
                                            for _ in range(2)]
                                    # softmax denominators accumulate on DVE
                                    # (bf16 running sum over tiles), freeing
                                    # a third of the inner-loop PE work
                                    acc = [accp.tile([128, QC], bf16,
                                                     tag=f"acc{a}", name=f"acc{a}")
                                           for a in range(2)]
                                    for g0, g1 in EBG:
                                        nt = g1 - g0
                                        ebg = atb.tile([128, 5 * QC], bf16,
                                                       tag="ebg", name="ebg")
                                        if JTS[g1 - 1][1] == 128:
                                            nc.sync.dma_start(
                                                out=ebg.rearrange(
                                                    "p (t n) -> p t n",
                                                    t=5)[:, :nt, :],
                                                in_=expB[
                                                    h,
                                                    JTS[g0][0]:JTS[g1 - 1][0] + 128,
                                                    qsl].rearrange(
                                                    "(t p) n -> p t n", p=128))
                                        else:
                                            nc.sync.dma_start(
                                                out=ebg[:JTS[g0][1], 0:QC],
                                                in_=expB[h, JTS[g0][0]:L, qsl])
                                        for i in range(g0, g1):
                                            j0, jn = JTS[i]
                                            eb = ebg[:jn, (i - g0) * QC:
                                                     (i - g0 + 1) * QC]
                                            for a, (kk, qq, vv) in enumerate(
                                                    ((kh, qh, vh), (kl, ql, vl))):
                                                ps_s = atps.tile([jn, QC], f32,
                                                                 tag="ps_s", name="ps_s")
                                                nc.tensor.matmul(
                                                    ps_s, kk[h][:, j0:j0 + jn],
                                                    qq[h][:, qsl],
                                                    start=True, stop=True)
                                                aa = at.tile([jn, QC], bf16, tag="aa", name="aa")
                                                nc.scalar.activation(
                                                    out=aa, in_=ps_s, func=AF.Exp,
                                                    bias=kbcol[(a, h)][:jn, i:i + 1],
                                                    scale=1.0)
                                                nc.vector.tensor_mul(aa, aa, eb)
                                                nc.tensor.matmul(
                                                    ps_o[a],
                                                    vv[i][:, h * 128:(h + 1) * 128],
                                                    aa, start=(i == 0),
                                                    stop=(i == len(JTS) - 1))
                                                if i == 0:
                                                    nc.vector.tensor_copy(
                                                        acc[a], aa)
                                                else:
                                                    nc.vector.tensor_add(
                                                        acc[a][:jn],
                                                        acc[a][:jn], aa)
                                    for a in range(2):
                                        ps_d = atpd.tile([128, QC], f32,
                                                         tag="ps_d", name="ps_d")
                                        nc.tensor.matmul(ps_d, ones_b, acc[a],
                                                         start=True, stop=True)
                                        rden = at.tile([128, QC], f32, tag="rden", name="rden")
                                        nc.vector.reciprocal_approx_fast(
                                            out=rden, in_=ps_d)
                                        on = ato.tile([128, QC], f32r,
                                                      tag=f"on{a}{h}", name=f"on{a}{h}")
                                        nc.vector.tensor_mul(on, ps_o[a], rden)
                                        onorm[(a, h)] = on
                                for a, (dst, tw, tb) in enumerate(
                                        ((oh, tfoh, bfoh), (ol, tfol, bfol))):
                                    for cb in range(CT):
                                        ps = atpp.tile([128, QC], f32,
                                                       tag="ps_fo", name="ps_fo")
                                        for h in range(HEADS):
                                            nc.tensor.matmul(
                                                ps, tw[h][:, cb * 128:(cb + 1) * 128],
                                                onorm[(a, h)],
                                                start=(h == 0),
                                                stop=(h == HEADS - 1))
                                        nc.vector.tensor_scalar_add(
                                            dst[cb][:, qsl], ps, tb[cb])

                # gate, mix, ff
                with tc.tile_pool(name="ph8w", bufs=1) as ph8w, \
                     tc.tile_pool(name="ph8", bufs=2) as ph8, \
                     tc.tile_pool(name="ph8p", bufs=2, space="PSUM") as ph8p:
                    tg1L = load_w3(ph8w, w_g1LT, "g1LT")
                    tg1R = load_w3(ph8w, w_g1RT, "g1RT")
                    bg1 = load_b3(ph8w, v_g1b, "g1b")
                    tg2 = load_w3(ph8w, w_g2T, "g2T")
                    g2b_t = ph8w.tile([1, 1], f32, tag="g2b_t", name="g2b_t")
                    nc.sync.dma_start(
                        out=g2b_t, in_=v_g2b.rearrange("(a b) -> a b", a=1))
                    tffL = load_w3(ph8w, w_ffLT, "ffLT")
                    tffP = load_w3(ph8w, w_ffPT, "ffPT")
                    bff = load_b3(ph8w, v_ffb, "ffb")
                    p1o = [ph8w.tile([128, QPC], bf16, tag=f"p1o{c}", name=f"p1o{c}")
                           for c in range(CT)]
                    for cb in range(CT):
                        nc.sync.dma_start(
                            out=p1o[cb],
                            in_=p1T_own[cb * 128:(cb + 1) * 128, :])
                    for qc in range(NQC):
                        qsl = slice(qc * QC, (qc + 1) * QC)
                        gel = []
                        for cb in range(CT):
                            ps = ph8p.tile([128, QC], f32, tag="ps_g1", name="ps_g1")
                            for kt in range(CT):
                                nc.tensor.matmul(
                                    ps, tg1L[kt][:, cb * 128:(cb + 1) * 128],
                                    oh[kt][:, qsl],
                                    start=(kt == 0), stop=False)
                            for kt in range(CT):
                                nc.tensor.matmul(
                                    ps, tg1R[kt][:, cb * 128:(cb + 1) * 128],
                                    ol[kt][:, qsl], start=False,
                                    stop=(kt == CT - 1))
                            gt = ph8.tile([128, QC], bf16, tag=f"ggel{cb}", name=f"ggel{cb}")
                            nc.scalar.activation(out=gt, in_=ps, func=AF.Gelu,
                                                 bias=bg1[cb], scale=1.0)
                            gel.append(gt)
                        ps_z = ph8p.tile([1, QC], f32, tag="ps_z", name="ps_z")
                        for kt in range(CT):
                            nc.tensor.matmul(ps_z, tg2[kt], gel[kt],
                                             start=(kt == 0),
                                             stop=(kt == CT - 1))
                        gate = ph8.tile([1, QC], f32r, tag="gate", name="gate")
                        nc.scalar.activation(out=gate, in_=ps_z,
                                             func=AF.Sigmoid,
                                             bias=g2b_t, scale=1.0)
                        ps_gb = ph8p.tile([128, QC], f32, tag="ps_gb", name="ps_gb")
                        nc.tensor.matmul(ps_gb, r32(ones_f[0:1, :]), gate,
                                         start=True, stop=True)
                        gb_bf = ph8.tile([128, QC], bf16, tag="gb_bf", name="gb_bf")
                        nc.vector.tensor_copy(gb_bf, ps_gb)
                        mix = []
                        for cb in range(CT):
                            dd = ph8.tile([128, QC], bf16, tag="dd", name="dd")
                            nc.vector.tensor_sub(dd, oh[cb][:, qsl],
                                                 ol[cb][:, qsl])
                            d2 = ph8.tile([128, QC], bf16, tag="d2", name="d2")
                            nc.vector.tensor_mul(d2, dd, gb_bf)
                            mx = ph8.tile([128, QC], bf16, tag=f"mix{cb}", name=f"mix{cb}")
                            nc.vector.tensor_add(mx, d2, ol[cb][:, qsl])
                            mix.append(mx)
                        for cb in range(CT):
                            ps = ph8p.tile([128, QC], f32, tag="ps_ff", name="ps_ff")
                            for kt in range(CT):
                                nc.tensor.matmul(
                                    ps, tffL[kt][:, cb * 128:(cb + 1) * 128],
                                    mix[kt], start=(kt == 0), stop=False)
                            for kt in range(CT):
                                nc.tensor.matmul(
                                    ps, tffP[kt][:, cb * 128:(cb + 1) * 128],
                                    p1o[kt][:, qsl], start=False,
                                    stop=(kt == CT - 1))
                            res = ph8.tile([128, QC], f32, tag="res", name="res")
                            nc.vector.tensor_scalar_add(res, ps, bff[cb])
                            nc.sync.dma_start(
                                out=outT[cb * 128:(cb + 1) * 128, qsl],
                                in_=res)

    nc.compile()
    return nc


def _prepare(inputs):
    """Host prep + input sharding. Returns (nc, in_maps)."""
    global _COMPILED
    import ml_dtypes
    bf = ml_dtypes.bfloat16
    inp = {k: np.asarray(v) for k, v in inputs.items()}
    g = _host_prep(inp)

    if _COMPILED is None:
        _COMPILED = _build()
    nc = _COMPILED

    p1 = inp["p1"].astype(np.float32)
    p2 = inp["p2"].astype(np.float32)

    # per-head scaled q-biases for the kb bias row (column layout)
    bq_h_col = g["bqh"].reshape(C, 1).astype(bf)
    bq_l_col = g["bql"].reshape(C, 1).astype(bf)

    shared = {
        "WupT": g["WupT"].astype(bf),
        "w_projT": g["projT"].astype(bf),
        "v_projb": g["projb"].reshape(1, C),
        "v_penw": g["penw"], "v_penb": g["penb"],
        "w_qhT": g["wqhT"].astype(bf), "v_bqh": g["bqh"],
        "w_qlT": g["wqlT"].astype(bf), "v_bql": g["bql"],
        "w_khT": g["wkhT"].astype(bf), "w_klT": g["wklT"].astype(bf),
        "w_vhT": g["wvhT"].astype(bf), "w_vlT": g["wvlT"].astype(bf),
        "bq_h_col": bq_h_col, "bq_l_col": bq_l_col,
        "w_pl1LT": g["pl1LT"].astype(bf), "w_pl1RT": g["pl1RT"].astype(bf),
        "v_pl1b": g["pl1b"],
        "w_pl2T": g["pl2T"].astype(bf), "v_pl2b": g["pl2b"],
        "w_fohT": g["fohT"], "v_fohb": g["fohb"],
        "w_folT": g["folT"], "v_folb": g["folb"],
        "w_g1LT": g["g1LT"].astype(bf), "w_g1RT": g["g1RT"].astype(bf),
        "v_g1b": g["g1b"],
        "w_g2T": g["g2T"].astype(bf), "v_g2b": g["g2b"],
        "w_ffLT": g["ffLT"].astype(bf), "w_ffPT": g["ffPT"].astype(bf),
        "v_ffb": g["ffb"],
    }
    shared = {k: np.ascontiguousarray(v) for k, v in shared.items()}

    in_maps = []
    for core in range(NCORES):
        b, qi = divmod(core, 4)
        q0 = qi * QPC
        m = dict(shared)
        m["p1T"] = np.ascontiguousarray(p1[b].T.astype(bf))
        m["p1T_own"] = np.ascontiguousarray(p1[b, q0:q0 + QPC, :].T.astype(bf))
        m["p2T"] = np.ascontiguousarray(p2[b].T.astype(bf))
        m["WupT_own"] = np.ascontiguousarray(g["WupT"][:, q0:q0 + QPC].astype(bf))
        m["expB"] = np.ascontiguousarray(g["expB"][:, :, q0:q0 + QPC])
        in_maps.append(m)

    return nc, in_maps


def _run(nc, in_maps):
    from concourse.bass_utils import run_bass_kernel_spmd
    res = run_bass_kernel_spmd(nc, in_maps, core_ids=list(range(NCORES)))
    out = np.zeros((B, L, C), np.float32)
    for core in range(NCORES):
        b, qi = divmod(core, 4)
        q0 = qi * QPC
        out[b, q0:q0 + QPC, :] = res.results[core]["outT"].T
    return out


def kernel(**inputs):
    nc, in_maps = _prepare(inputs)
    return _run(nc, in_maps)
